# revision 38
# baseline (speedup 1.0000x reference)
"""Trainium2 Bass kernel for nn_FreqCrossAttention.

Sharding: 8 cores = 4 batches x 2 head-groups (8 heads each).
Each core computes a partial output [2048, 1024] (its head-group's
contribution through W_o row-parallel); host sums the pair per batch.

v3: radix-2 DFT. Host permutes the L axis into (even, odd) halves; the
rfft becomes two 1024-point half-DFTs (twiddles folded into the odd
basis matrices) plus a 4-op combine:
    A (freqs k=0..512)        = E + P
    B (freqs k=1024..513)     : Br = Er - Pr, Bi = Pi - Ei
Frequencies are stored in "split order" [A | B]; the inverse
permutation is folded into the host-built iDFT matrices. Gamma is
folded into Wqr/Wqi host-side; beta's DC term is added on-device to
storage column u=0 post-projection. DFT matmul FLOPs halve.
"""
import math
import numpy as np
import ml_dtypes

MM_BF16 = True

B, L, E, H = 4, 2048, 1024, 16
D = E // H            # 64
Lf = L // 2 + 1       # 1025
FP = 1026             # padded frequency dim (split-order + 1 pad col)
NH = 8                # heads per core
P = 128
LH = 8                # L-tiles per half (even/odd)
JCH = [(0, 256), (256, 257)]          # j-chunks of the 513 half-freqs
JOFF = [0, LH * 256]                  # free offsets in pretiled slabs
# storage chunks: (u0, width, j-chunk index, block)
SCH = [(0, 256, 0, 'A'), (256, 257, 1, 'A'),
       (513, 256, 0, 'B'), (769, 256, 1, 'B')]
# m-tiles in storage order (1025 real bins; singleton at u=512);
# also used as l-tiles for AV and j-tiles for the radix-2 iDFT
MTI = [(0, 128), (128, 128), (256, 128), (384, 128), (512, 1),
       (513, 128), (641, 128), (769, 128), (897, 128)]
FCH = [(0, 384), (384, 384), (768, 257)]   # scores moving-dim chunks
ET = 8                # e-chunks of E
LT = 16               # L tiles
EPS = 1e-5

_CACHE = {}


def _pretile_half(M):
    # [1024, 513] -> [P, LH*513] chunk-major (j-chunks of JCH)
    r = M.reshape(LH, P, 513)
    blocks = []
    for (j0, jsz) in JCH:
        blocks.append(np.ascontiguousarray(
            r[:, :, j0:j0 + jsz].transpose(1, 0, 2).reshape(P, LH * jsz)))
    return np.concatenate(blocks, axis=1)              # [P, LH*513]


def _dft_consts():
    s = 1.0 / math.sqrt(L)
    m = np.arange(1024)[:, None].astype(np.float64)
    j = np.arange(513)[None, :].astype(np.float64)
    ang_e = 2.0 * np.pi * (2.0 * m) * j / L
    ang_o = 2.0 * np.pi * (2.0 * m + 1.0) * j / L
    Ce = (np.cos(ang_e) * s).astype(np.float32)
    Se = (-np.sin(ang_e) * s).astype(np.float32)
    Co = (np.cos(ang_o) * s).astype(np.float32)
    So = (-np.sin(ang_o) * s).astype(np.float32)
    # radix-2 iDFT half-matrices (cw doubling pre-folded: rows x2; the
    # DC/Nyquist rows of our/oui get pre-halved on device instead)
    tau = np.arange(1024)[None, :].astype(np.float64)
    jj = np.arange(513)[:, None].astype(np.float64)
    Ec = (2.0 * s * np.cos(2.0 * np.pi * jj * tau / 1024.0)).astype(np.float32)
    Es = (-2.0 * s * np.sin(2.0 * np.pi * jj * tau / 1024.0)).astype(np.float32)
    Oc = (2.0 * s * np.cos(np.pi * jj * (2.0 * tau + 1.0) / 1024.0)).astype(np.float32)
    Os = (-2.0 * s * np.sin(np.pi * jj * (2.0 * tau + 1.0) / 1024.0)).astype(np.float32)
    return Ce, Se, Co, So, Ec, Es, Oc, Os


def _build():
    import concourse.bass as bass
    import concourse.bacc as bacc
    import concourse.mybir as mybir
    import concourse.tile as tile

    R = mybir.dt.bfloat16 if MM_BF16 else mybir.dt.float32r
    F32 = mybir.dt.float32
    BF16 = mybir.dt.bfloat16
    AF = mybir.ActivationFunctionType

    nc = bacc.Bacc("TRN2", debug=False, num_devices=8)

    q_d = nc.dram_tensor("q", [L, E], F32, kind="ExternalInput")
    kvt_d = nc.dram_tensor("kvt", [ET * P, LT * P], R, kind="ExternalInput")
    slab_d = {}
    for nm in ("ce", "se", "co", "so"):
        slab_d[nm] = nc.dram_tensor(nm, [P, LH * 513], R, kind="ExternalInput")
    ig_d = {}
    for nm in ("iec", "ies", "ioc", "ios"):
        ig_d[nm] = nc.dram_tensor(nm, [513, 1024], R, kind="ExternalInput")
    W_d = {}
    for nm in ("qr", "qi", "kr", "ki", "vr", "vi"):
        W_d[nm] = nc.dram_tensor(f"W{nm}", [E, 512], R, kind="ExternalInput")
        W_d["b" + nm] = nc.dram_tensor(f"b{nm}", [512, 1], F32, kind="ExternalInput")
    dc_d = {nm: nc.dram_tensor(f"dc{nm}", [512, 1], F32, kind="ExternalInput")
            for nm in ("qr", "qi")}
    WoT_d = nc.dram_tensor("WoT", [512, E], R, kind="ExternalInput")
    out_d = nc.dram_tensor("out", [L, E], F32, kind="ExternalOutput")

    with tile.TileContext(nc) as tc:
        with tc.tile_pool(name="dram", bufs=1, space="DRAM") as dram, \
             tc.tile_pool(name="persist", bufs=1) as persist, \
             tc.tile_pool(name="kcl", bufs=1) as kcl, \
             tc.tile_pool(name="vcl", bufs=1) as vcl, \
             tc.tile_pool(name="oacc", bufs=1) as oacc:
            Qcat_dram = dram.tile([NH, P, FP], R)

            eps_t = persist.tile([P, 1], F32)
            nc.vector.memset(eps_t[:], EPS)

            # K and V stay SBUF-resident from projection through attention
            Kc = [kcl.tile([P, FP], R, tag=f"Kc{h}", name=f"Kc{h}")
                  for h in range(NH)]
            Vc = [vcl.tile([P, NH * 129], BF16, tag=f"Vc{ti}", name=f"Vc{ti}")
                  for ti in range(len(MTI))]

            our = []
            oui = []
            for ti in range(len(MTI)):
                our.append(oacc.tile([P, 512], R, tag=f"our{ti}", name=f"our{ti}"))
                oui.append(oacc.tile([P, 512], R, tag=f"oui{ti}", name=f"oui{ti}"))

            # ---------- radix-2 DFT + projection worker ----------
            def dft_proj(is_q, get_xe, Wt, bias_t, vbias, slabs, dps, qfp, stg, dct):
                pnames = ("qr", "qi") if is_q else ("kr", "ki")
                for jc, (j0, jsz) in enumerate(JCH):
                    bw = 256   # B-block width in this chunk
                    xa_r, xa_i, xb_r, xb_i = [], [], [], []
                    for eb in range(ET):
                        xe = get_xe(eb)
                        pEr = dps.tile([P, jsz], F32, tag="pEr", name="pEr")
                        pEi = dps.tile([P, jsz], F32, tag="pEi", name="pEi")
                        pPr = dps.tile([P, jsz], F32, tag="pPr", name="pPr")
                        pPi = dps.tile([P, jsz], F32, tag="pPi", name="pPi")
                        for lh in range(LH):
                            sl = slice(lh * jsz, (lh + 1) * jsz)
                            st_, sp_ = (lh == 0), (lh == LH - 1)
                            nc.tensor.matmul(pEr[:], xe(lh), slabs["ce"][jc][:, sl],
                                             start=st_, stop=sp_)
                            nc.tensor.matmul(pEi[:], xe(lh), slabs["se"][jc][:, sl],
                                             start=st_, stop=sp_)
                            nc.tensor.matmul(pPr[:], xe(LH + lh), slabs["co"][jc][:, sl],
                                             start=st_, stop=sp_)
                            nc.tensor.matmul(pPi[:], xe(LH + lh), slabs["so"][jc][:, sl],
                                             start=st_, stop=sp_)
                        # E -> SBUF via ACT, then 4 DVE combines (SBUF, PSUM)
                        erS = stg.tile([P, 257], F32, tag="erS", name="erS", bufs=2)
                        eiS = stg.tile([P, 257], F32, tag="eiS", name="eiS", bufs=2)
                        nc.scalar.activation(erS[:, 0:jsz], pEr[:], AF.Copy)
                        nc.scalar.activation(eiS[:, 0:jsz], pEi[:], AF.Copy)
                        ar = qfp.tile([P, jsz], R, tag=f"ar{eb}", name=f"ar{eb}")
                        ai = qfp.tile([P, jsz], R, tag=f"ai{eb}", name=f"ai{eb}")
                        br = qfp.tile([P, bw], R, tag=f"br{eb}", name=f"br{eb}")
                        bi = qfp.tile([P, bw], R, tag=f"bi{eb}", name=f"bi{eb}")
                        nc.vector.tensor_add(ar[:], erS[:, 0:jsz], pPr[:])
                        nc.vector.tensor_add(ai[:], eiS[:, 0:jsz], pPi[:])
                        nc.vector.tensor_sub(br[:], erS[:, 0:bw], pPr[:, 0:bw])
                        nc.vector.tensor_sub(bi[:], pPi[:, 0:bw], eiS[:, 0:bw])
                        xa_r.append(ar)
                        xa_i.append(ai)
                        xb_r.append(br)
                        xb_i.append(bi)

                    for (u0, w, jcc, kind) in SCH:
                        if jcc != jc:
                            continue
                        xr = xa_r if kind == 'A' else xb_r
                        xi = xa_i if kind == 'A' else xb_i
                        # Q/K projections for this storage chunk
                        for mt in range(4):
                            pps = {}
                            for nm in pnames:
                                pps[nm] = dps.tile([P, w], F32, tag=f"pp{nm}", name=f"pp{nm}")
                            for ec in range(ET):
                                src = {pnames[0]: xr[ec], pnames[1]: xi[ec]}
                                for nm in pnames:
                                    nc.tensor.matmul(pps[nm][:],
                                                     Wt[nm][ec][:, mt * P:(mt + 1) * P],
                                                     src[nm][:],
                                                     start=(ec == 0), stop=(ec == ET - 1))
                            sg = {}
                            for nm in pnames:
                                s = stg.tile([P, 257], R, tag=f"sg{nm}", name=f"sg{nm}", bufs=2)
                                nc.scalar.activation(s[:, 0:w], pps[nm][:], AF.Identity,
                                                     bias=bias_t[nm][mt][:])
                                if is_q and u0 == 0:
                                    # beta's DC contribution (storage col u=0)
                                    nc.vector.scalar_tensor_tensor(
                                        s[:, 0:1], dct[nm][mt][:], 1.0,
                                        s[:, 0:1],
                                        op0=mybir.AluOpType.mult,
                                        op1=mybir.AluOpType.add)
                                sg[nm] = s
                            r0, i0 = pnames
                            if is_q:
                                nc.sync.dma_start(Qcat_dram[2 * mt, 0:64, u0:u0 + w], sg[r0][0:64, 0:w])
                                nc.sync.dma_start(Qcat_dram[2 * mt + 1, 0:64, u0:u0 + w], sg[r0][64:128, 0:w])
                                nc.sync.dma_start(Qcat_dram[2 * mt, 64:128, u0:u0 + w], sg[i0][0:64, 0:w])
                                nc.sync.dma_start(Qcat_dram[2 * mt + 1, 64:128, u0:u0 + w], sg[i0][64:128, 0:w])
                            else:
                                # SBUF->SBUF partition-shuffling DMAs into Kc
                                nc.sync.dma_start(Kc[2 * mt][0:64, u0:u0 + w], sg[r0][0:64, 0:w])
                                nc.sync.dma_start(Kc[2 * mt + 1][0:64, u0:u0 + w], sg[r0][64:128, 0:w])
                                nc.sync.dma_start(Kc[2 * mt][64:128, u0:u0 + w], sg[i0][0:64, 0:w])
                                nc.sync.dma_start(Kc[2 * mt + 1][64:128, u0:u0 + w], sg[i0][64:128, 0:w])

                        # V projection (kv path only) -> directly into Vc
                        if not is_q:
                            for vti, (m0, msz) in enumerate(MTI):
                                if not (m0 >= u0 and m0 + msz <= u0 + w):
                                    continue
                                mr = m0 - u0
                                pvr = dps.tile([P, 512], F32, tag="pvr", name="pvr")
                                pvi = dps.tile([P, 512], F32, tag="pvi", name="pvi")
                                for ec in range(ET):
                                    nc.tensor.matmul(pvr[0:msz, :], xr[ec][:, mr:mr + msz],
                                                     Wt["vr"][ec][:],
                                                     start=(ec == 0), stop=(ec == ET - 1))
                                    nc.tensor.matmul(pvi[0:msz, :], xi[ec][:, mr:mr + msz],
                                                     Wt["vi"][ec][:],
                                                     start=(ec == 0), stop=(ec == ET - 1))
                                vco = Vc[vti][0:msz, :].rearrange("p (h c) -> p h c", h=NH)
                                nc.vector.tensor_add(
                                    vco[:, :, 0:64],
                                    pvr[0:msz, :].rearrange("p (h c) -> p h c", h=NH),
                                    vbias["vr"][0:msz, :].rearrange("p (h c) -> p h c", h=NH))
                                nc.vector.tensor_add(
                                    vco[:, :, 64:128],
                                    pvi[0:msz, :].rearrange("p (h c) -> p h c", h=NH),
                                    vbias["vi"][0:msz, :].rearrange("p (h c) -> p h c", h=NH))
                                nc.vector.memset(vco[:, :, 128:129], 1.0)

            # ================= Phase 1 =================
            with tc.tile_pool(name="qn", bufs=1) as qnp, \
                 tc.tile_pool(name="fsl", bufs=1) as fslp, \
                 tc.tile_pool(name="qf", bufs=1) as qfp, \
                 tc.tile_pool(name="stg", bufs=2) as stg:
                # DFT basis slabs (shared by kv and q paths), loaded once,
                # split per j-chunk so only jc0 halves gate the first MMs
                slabs = {nm: [None, None] for nm in ("ce", "se", "co", "so")}

                def load_slabs(jc):
                    j0, jsz = JCH[jc]
                    for nm in ("ce", "se", "co", "so"):
                        t = fslp.tile([P, LH * jsz], R, tag=f"{nm}{jc}", name=f"{nm}{jc}")
                        nc.sync.dma_start(
                            t[:], slab_d[nm].ap()[:, JOFF[jc]:JOFF[jc] + LH * jsz])
                        slabs[nm][jc] = t

                qn_t = [qnp.tile([P, E], R, tag=f"qn{lc}", name=f"qn{lc}")
                        for lc in range(LT)]

                # ---- Phase 1a: kv path (kv tiles streamed per (jc, eb)) ----
                with tc.tile_pool(name="kvs", bufs=1) as kvp, \
                     tc.tile_pool(name="wkv", bufs=1) as wkv, \
                     tc.tile_pool(name="dpsa", bufs=1, space="PSUM") as dpsa:
                    def load_kvt(eb):
                        t = kvp.tile([P, LT * P], R, tag=f"kvs{eb % 3}",
                                     name=f"kvs{eb % 3}")
                        nc.sync.dma_start(t[:], kvt_d.ap()[eb * P:(eb + 1) * P, :])
                        return t
                    # startup-critical DMA order: kvt0, jc0 slabs, kvt1-2,
                    # jc1 slabs, then weights
                    kv_pre = {0: load_kvt(0)}
                    load_slabs(0)
                    kv_pre[1] = load_kvt(1)
                    kv_pre[2] = load_kvt(2)
                    load_slabs(1)

                    def get_xe_kv(eb):
                        t = kv_pre.pop(eb) if eb in kv_pre else load_kvt(eb)
                        return lambda lc: t[:, lc * P:(lc + 1) * P]
                    Wt = {}
                    bias_t = {}
                    for nm in ("kr", "ki", "vr", "vi"):
                        Wt[nm] = [wkv.tile([P, 512], R, tag=f"W{nm}{ec}", name=f"W{nm}{ec}")
                                  for ec in range(ET)]
                        for ec in range(ET):
                            nc.sync.dma_start(Wt[nm][ec][:],
                                                W_d[nm].ap()[ec * P:(ec + 1) * P, :])
                        if nm in ("kr", "ki"):
                            bias_t[nm] = [wkv.tile([P, 1], F32, tag=f"b{nm}{mt}", name=f"b{nm}{mt}")
                                          for mt in range(4)]
                            for mt in range(4):
                                nc.sync.dma_start(bias_t[nm][mt][:],
                                                    W_d["b" + nm].ap()[mt * P:(mt + 1) * P, :])
                    vb_row = wkv.tile([1, 512], F32, tag="vbrow", name="vbrow")
                    vbias = {}
                    for nm in ("vr", "vi"):
                        nc.sync.dma_start(vb_row[:], W_d["b" + nm].ap().rearrange("e one -> one e"))
                        vb = wkv.tile([P, 512], F32, tag=f"vb{nm}", name=f"vb{nm}")
                        nc.gpsimd.partition_broadcast(vb[:], vb_row[:])
                        vbias[nm] = vb

                    dft_proj(False, get_xe_kv,
                             Wt, bias_t, vbias, slabs, dpsa, qfp, stg, None)

                    # LN of q (emitted after kv work: q DMAs queue behind
                    # kv-phase loads; DVE/ACT fill in around kv evictions)
                    with tc.tile_pool(name="ln", bufs=2) as ln, \
                         tc.tile_pool(name="lns", bufs=4) as lns:
                        for lc in range(LT):
                            qt = ln.tile([P, E], F32, tag="qt", name="qt")
                            nc.sync.dma_start(qt[:], q_d.ap()[lc * P:(lc + 1) * P, :])
                            st = lns.tile([P, 12], F32, tag="st", name="st")
                            nc.vector.bn_stats(st[:, 0:6], qt[:, 0:512])
                            nc.vector.bn_stats(st[:, 6:12], qt[:, 512:1024])
                            mv = lns.tile([P, 2], F32, tag="mv", name="mv")
                            nc.vector.bn_aggr(mv[:], st[:])
                            sd = lns.tile([P, 1], F32, tag="sd", name="sd")
                            nc.scalar.activation(sd[:], mv[:, 1:2], AF.Sqrt, bias=eps_t[:])
                            istd = lns.tile([P, 1], F32, tag="istd", name="istd")
                            nc.vector.reciprocal(istd[:], sd[:])
                            nmu = lns.tile([P, 1], F32, tag="nmu", name="nmu")
                            nc.vector.tensor_scalar_mul(nmu[:], mv[:, 0:1], -1.0)
                            nc.vector.tensor_mul(nmu[:], nmu[:], istd[:])
                            nc.scalar.activation(qn_t[lc][:], qt[:], AF.Identity,
                                                 bias=nmu[:], scale=istd[:])

                # ---- Phase 1b: q path ----
                with tc.tile_pool(name="wq", bufs=1) as wq, \
                     tc.tile_pool(name="dpsb", bufs=1, space="PSUM") as dpsb:
                    Wtq = {}
                    bias_q = {}
                    dct = {}
                    for nm in ("qr", "qi"):
                        Wtq[nm] = [wq.tile([P, 512], R, tag=f"W{nm}{ec}", name=f"W{nm}{ec}")
                                   for ec in range(ET)]
                        for ec in range(ET):
                            nc.sync.dma_start(Wtq[nm][ec][:],
                                                W_d[nm].ap()[ec * P:(ec + 1) * P, :])
                        bias_q[nm] = [wq.tile([P, 1], F32, tag=f"b{nm}{mt}", name=f"b{nm}{mt}")
                                      for mt in range(4)]
                        dct[nm] = [wq.tile([P, 1], F32, tag=f"dc{nm}{mt}", name=f"dc{nm}{mt}")
                                   for mt in range(4)]
                        for mt in range(4):
                            nc.sync.dma_start(bias_q[nm][mt][:],
                                                W_d["b" + nm].ap()[mt * P:(mt + 1) * P, :])
                            nc.sync.dma_start(dct[nm][mt][:],
                                                dc_d[nm].ap()[mt * P:(mt + 1) * P, :])
                    dft_proj(True,
                             lambda eb: (lambda lc: qn_t[lc][:, eb * P:(eb + 1) * P]),
                             Wtq, bias_q, None, slabs, dpsb, qfp, stg, dct)

            # ================= Phase 2: attention =================
            attn_ctx = [tc.tile_pool(name="qk", bufs=1),
                        tc.tile_pool(name="expp", bufs=3),
                        tc.tile_pool(name="sps", bufs=4, space="PSUM"),
                        tc.tile_pool(name="avps", bufs=3, space="PSUM"),
                        tc.tile_pool(name="nrm", bufs=4)]
            qk, expp, sps, avps, nrm = [c.__enter__() for c in attn_ctx]
            Qc = []
            for h in range(NH):
                qt = qk.tile([P, FP], R, tag=f"Qc{h}", name=f"Qc{h}")
                nc.sync.dma_start(qt[:], Qcat_dram[h, :, :])
                Qc.append(qt)

            def do_av(h, expts):
                for ti, (l0, lsz) in enumerate(MTI):
                    ps = avps.tile([P, 129], F32, tag="av", name="av")
                    n = len(MTI)
                    for mi, (m0, msz) in enumerate(MTI):
                        nc.tensor.matmul(ps[0:lsz, :], expts[mi][0:msz, l0:l0 + lsz],
                                         Vc[mi][0:msz, h * 129:(h + 1) * 129],
                                         start=(mi == 0), stop=(mi == n - 1))
                    rcp = nrm.tile([P, 1], F32, tag="rcp", name="rcp")
                    nc.vector.reciprocal(rcp[0:lsz, :], ps[0:lsz, 128:129])
                    nc.vector.tensor_scalar_mul(our[ti][0:lsz, h * 64:(h + 1) * 64],
                                                ps[0:lsz, 0:64], rcp[0:lsz, :])
                    nc.vector.tensor_scalar_mul(oui[ti][0:lsz, h * 64:(h + 1) * 64],
                                                ps[0:lsz, 64:128], rcp[0:lsz, :])

            # software pipeline: AV for head h-1 overlaps scores/exp for h
            prev = None
            for h in range(NH):
                expts = []
                for ti, (m0, msz) in enumerate(MTI):
                    et_ = expp.tile([P, FP], BF16, tag=f"exp{ti}", name=f"exp{ti}")
                    for (f0, fsz) in FCH:
                        ps = sps.tile([P, 384], F32, tag="sc", name="sc")
                        nc.tensor.matmul(ps[0:msz, 0:fsz], Kc[h][:, m0:m0 + msz],
                                         Qc[h][:, f0:f0 + fsz], start=True, stop=True)
                        nc.scalar.activation(et_[0:msz, f0:f0 + fsz], ps[0:msz, 0:fsz],
                                             AF.Exp, scale=float(D ** -0.5))
                    expts.append(et_)
                if prev is not None:
                    do_av(h - 1, prev)
                prev = expts
            do_av(NH - 1, prev)
            for c in reversed(attn_ctx):
                c.__exit__(None, None, None)

            # ================= Phase 3: radix-2 iDFT + Wo =================
            # OTT columns hold [even times tau 0..1023 | odd times]; the
            # final output DMA de-interleaves via a strided DRAM view.
            with tc.tile_pool(name="gsl", bufs=3) as gsl, \
                 tc.tile_pool(name="uvp", bufs=1) as uvp, \
                 tc.tile_pool(name="ott", bufs=1) as ottp, \
                 tc.tile_pool(name="wop", bufs=1) as wop, \
                 tc.tile_pool(name="ost", bufs=3) as ost:
                OTT = [ottp.tile([P, L], R, tag=f"OTT{i}", name=f"OTT{i}") for i in range(4)]
                WoT_t = [wop.tile([P, E], R, tag=f"wo{i}", name=f"wo{i}") for i in range(4)]
                for ec in range(4):
                    nc.sync.dma_start(WoT_t[ec][:], WoT_d.ap()[ec * P:(ec + 1) * P, :])
                # pre-halve DC (u=0) and Nyquist (u=513 -> k=1024) rows;
                # basis matrices carry the x2 weighting for all rows
                for t_ in (our[0], oui[0], our[5], oui[5]):
                    nc.vector.tensor_scalar_mul(t_[0:1, :], t_[0:1, :], 0.5)
                Ur, Ui, Vr, Vi = [], [], [], []
                for jt in range(4):
                    u_r = uvp.tile([P, 512], R, tag=f"Ur{jt}", name=f"Ur{jt}")
                    u_i = uvp.tile([P, 512], R, tag=f"Ui{jt}", name=f"Ui{jt}")
                    v_r = uvp.tile([P, 512], R, tag=f"Vr{jt}", name=f"Vr{jt}")
                    v_i = uvp.tile([P, 512], R, tag=f"Vi{jt}", name=f"Vi{jt}")
                    nc.vector.tensor_add(u_r[:], our[jt][:], our[5 + jt][:])
                    nc.vector.tensor_sub(v_r[:], our[jt][:], our[5 + jt][:])
                    nc.vector.tensor_sub(u_i[:], oui[jt][:], oui[5 + jt][:])
                    nc.vector.tensor_add(v_i[:], oui[jt][:], oui[5 + jt][:])
                    Ur.append(u_r)
                    Ui.append(u_i)
                    Vr.append(v_r)
                    Vi.append(v_i)

                out_v = out_d.ap().rearrange("(t two) e -> two t e", two=2)
                idps_ctx = tc.tile_pool(name="idps", bufs=1, space="PSUM")
                idps = idps_ctx.__enter__()
                wops_ctx = tc.tile_pool(name="wops", bufs=2, space="PSUM")
                wops = wops_ctx.__enter__()

                def wo_block(tb):
                    pso = [wops.tile([P, 512], F32, tag=f"po{eo}", name=f"po{eo}")
                           for eo in range(2)]
                    for eo in range(2):
                        for ec in range(4):
                            nc.tensor.matmul(pso[eo][:],
                                             OTT[ec][:, tb * P:(tb + 1) * P],
                                             WoT_t[ec][:, eo * 512:(eo + 1) * 512],
                                             start=(ec == 0), stop=(ec == 3))
                    ot_ = ost.tile([P, E], F32, tag="ot", name="ot")
                    nc.vector.tensor_copy(ot_[:, 0:512], pso[0][:])
                    nc.scalar.activation(ot_[:, 512:1024], pso[1][:], AF.Copy)
                    nc.sync.dma_start(
                        out_v[tb // 8, (tb % 8) * P:(tb % 8 + 1) * P, :], ot_[:])

                # 4-bank iDFT passes (oh x e4-pair) so Wo for the even half
                # overlaps the odd-half iDFT on the PE
                for oh in range(2):
                    nm_c, nm_s = ("iec", "ies") if oh == 0 else ("ioc", "ios")
                    for ep in range(2):
                        pst = [[idps.tile([P, 512], F32, tag=f"ph{i}_{t2}", name=f"ph{i}_{t2}")
                                for t2 in range(2)] for i in range(2)]
                        for jt in range(5):
                            msz = P if jt < 4 else 1
                            r0 = jt * P
                            mc = gsl.tile([P, 1024], R, tag="gc", name="gc")
                            ms = gsl.tile([P, 1024], R, tag="gs", name="gs")
                            nc.sync.dma_start(mc[0:msz, :], ig_d[nm_c].ap()[r0:r0 + msz, :])
                            nc.sync.dma_start(ms[0:msz, :], ig_d[nm_s].ap()[r0:r0 + msz, :])
                            if jt < 4:
                                sr = Ur[jt] if oh == 0 else Vr[jt]
                                si = Ui[jt] if oh == 0 else Vi[jt]
                            else:
                                sr, si = our[4], oui[4]
                            for i in range(2):
                                e4 = 2 * ep + i
                                for t2 in range(2):
                                    nc.tensor.matmul(pst[i][t2][:],
                                                     sr[0:msz, e4 * P:(e4 + 1) * P],
                                                     mc[0:msz, t2 * 512:(t2 + 1) * 512],
                                                     start=(jt == 0), stop=False)
                                    nc.tensor.matmul(pst[i][t2][:],
                                                     si[0:msz, e4 * P:(e4 + 1) * P],
                                                     ms[0:msz, t2 * 512:(t2 + 1) * 512],
                                                     start=False, stop=(jt == 4))
                        for i in range(2):
                            e4 = 2 * ep + i
                            for t2 in range(2):
                                dst = OTT[e4][:, oh * 1024 + t2 * 512:oh * 1024 + (t2 + 1) * 512]
                                if i == 0:
                                    nc.vector.tensor_copy(dst, pst[i][t2][:])
                                else:
                                    nc.scalar.activation(dst, pst[i][t2][:], AF.Copy)
                    if oh == 0:
                        for tb in range(8):
                            wo_block(tb)
                for tb in range(8, LT):
                    wo_block(tb)
                wops_ctx.__exit__(None, None, None)
                idps_ctx.__exit__(None, None, None)

    nc.finalize()
    return nc


def kernel(**inputs):
    from concourse.bass_utils import run_bass_kernel_spmd

    rdt = ml_dtypes.bfloat16 if MM_BF16 else np.float32
    if "nc" not in _CACHE:
        _CACHE["nc"] = _build()
        Ce, Se, Co, So, Ec, Es, Oc, Os = _dft_consts()
        _CACHE["consts"] = {
            "ce": _pretile_half(Ce).astype(rdt),
            "se": _pretile_half(Se).astype(rdt),
            "co": _pretile_half(Co).astype(rdt),
            "so": _pretile_half(So).astype(rdt),
            "iec": Ec.astype(rdt), "ies": Es.astype(rdt),
            "ioc": Oc.astype(rdt), "ios": Os.astype(rdt),
        }
    nc = _CACHE["nc"]
    C = _CACHE["consts"]

    perm = np.r_[0:L:2, 1:L:2]
    q = np.ascontiguousarray(inputs["query"], dtype=np.float32)[:, perm, :]
    kv = np.asarray(inputs["key_value"], dtype=np.float32)[:, perm, :]
    gamma = np.asarray(inputs["gamma"], np.float32)
    beta = np.asarray(inputs["beta"], np.float32)
    in_maps = []
    for core in range(8):
        b = core // 2
        hg = core % 2
        cs = slice(hg * 512, (hg + 1) * 512)
        kvt = np.ascontiguousarray(
            kv[b].reshape(LT, P, ET, P).transpose(2, 1, 0, 3).reshape(ET * P, LT * P)
        ).astype(rdt)
        m = {
            "q": np.ascontiguousarray(q[b]),
            "kvt": kvt,
            "ce": C["ce"], "se": C["se"], "co": C["co"], "so": C["so"],
            "iec": C["iec"], "ies": C["ies"], "ioc": C["ioc"], "ios": C["ios"],
            "WoT": np.ascontiguousarray(inputs["Wo"][:, cs].T.astype(rdt)),
        }
        for nm in ("qr", "qi", "kr", "ki", "vr", "vi"):
            Wcs = np.asarray(inputs["W" + nm], np.float32)[cs, :]
            if nm in ("qr", "qi"):
                m[f"dc{nm}"] = np.ascontiguousarray(
                    (Wcs @ beta) * math.sqrt(L), np.float32).reshape(512, 1)
                Wcs = Wcs * gamma[None, :]
            m[f"W{nm}"] = np.ascontiguousarray(Wcs.T.astype(rdt))
            m[f"b{nm}"] = np.ascontiguousarray(inputs["b" + nm][cs], np.float32).reshape(512, 1)
        in_maps.append(m)

    res = run_bass_kernel_spmd(nc, in_maps, core_ids=list(range(8)))
    _CACHE["last"] = res
    out = np.empty((B, L, E), np.float32)
    for b in range(B):
        out[b] = res.results[2 * b]["out"] + res.results[2 * b + 1]["out"]
    return out


# revision 39
# speedup vs baseline: 1.0252x; 1.0252x over previous
"""Trainium2 Bass kernel for nn_FreqCrossAttention.

Sharding: 8 cores = 4 batches x 2 head-groups (8 heads each).
Each core computes a partial output [2048, 1024] (its head-group's
contribution through W_o row-parallel); host sums the pair per batch.

v3: radix-2 DFT. Host permutes the L axis into (even, odd) halves; the
rfft becomes two 1024-point half-DFTs (twiddles folded into the odd
basis matrices) plus a 4-op combine:
    A (freqs k=0..512)        = E + P
    B (freqs k=1024..513)     : Br = Er - Pr, Bi = Pi - Ei
Frequencies are stored in "split order" [A | B]; the inverse
permutation is folded into the host-built iDFT matrices. Gamma is
folded into Wqr/Wqi host-side; beta's DC term is added on-device to
storage column u=0 post-projection. DFT matmul FLOPs halve.
"""
import math
import numpy as np
import ml_dtypes

MM_BF16 = True

B, L, E, H = 4, 2048, 1024, 16
D = E // H            # 64
Lf = L // 2 + 1       # 1025
FP = 1026             # padded frequency dim (split-order + 1 pad col)
NH = 8                # heads per core
P = 128
LH = 8                # L-tiles per half (even/odd)
JCH = [(0, 256), (256, 257)]          # j-chunks of the 513 half-freqs
JOFF = [0, LH * 256]                  # free offsets in pretiled slabs
# storage chunks: (u0, width, j-chunk index, block)
SCH = [(0, 256, 0, 'A'), (256, 257, 1, 'A'),
       (513, 256, 0, 'B'), (769, 256, 1, 'B')]
# m-tiles in storage order (1025 real bins; singleton at u=512);
# also used as l-tiles for AV and j-tiles for the radix-2 iDFT
MTI = [(0, 128), (128, 128), (256, 128), (384, 128), (512, 1),
       (513, 128), (641, 128), (769, 128), (897, 128)]
FCH = [(0, 384), (384, 384), (768, 257)]   # scores moving-dim chunks
ET = 8                # e-chunks of E
LT = 16               # L tiles
EPS = 1e-5

_CACHE = {}


def _pretile_half(M):
    # [1024, 513] -> [P, LH*513] chunk-major (j-chunks of JCH)
    r = M.reshape(LH, P, 513)
    blocks = []
    for (j0, jsz) in JCH:
        blocks.append(np.ascontiguousarray(
            r[:, :, j0:j0 + jsz].transpose(1, 0, 2).reshape(P, LH * jsz)))
    return np.concatenate(blocks, axis=1)              # [P, LH*513]


def _dft_consts():
    s = 1.0 / math.sqrt(L)
    m = np.arange(1024)[:, None].astype(np.float64)
    j = np.arange(513)[None, :].astype(np.float64)
    ang_e = 2.0 * np.pi * (2.0 * m) * j / L
    ang_o = 2.0 * np.pi * (2.0 * m + 1.0) * j / L
    Ce = (np.cos(ang_e) * s).astype(np.float32)
    Se = (-np.sin(ang_e) * s).astype(np.float32)
    Co = (np.cos(ang_o) * s).astype(np.float32)
    So = (-np.sin(ang_o) * s).astype(np.float32)
    # radix-2 iDFT half-matrices (cw doubling pre-folded: rows x2; the
    # DC/Nyquist rows of our/oui get pre-halved on device instead)
    tau = np.arange(1024)[None, :].astype(np.float64)
    jj = np.arange(513)[:, None].astype(np.float64)
    Ec = (2.0 * s * np.cos(2.0 * np.pi * jj * tau / 1024.0)).astype(np.float32)
    Es = (-2.0 * s * np.sin(2.0 * np.pi * jj * tau / 1024.0)).astype(np.float32)
    Oc = (2.0 * s * np.cos(np.pi * jj * (2.0 * tau + 1.0) / 1024.0)).astype(np.float32)
    Os = (-2.0 * s * np.sin(np.pi * jj * (2.0 * tau + 1.0) / 1024.0)).astype(np.float32)
    return Ce, Se, Co, So, Ec, Es, Oc, Os


def _build():
    import concourse.bass as bass
    import concourse.bacc as bacc
    import concourse.mybir as mybir
    import concourse.tile as tile

    R = mybir.dt.bfloat16 if MM_BF16 else mybir.dt.float32r
    F32 = mybir.dt.float32
    BF16 = mybir.dt.bfloat16
    AF = mybir.ActivationFunctionType

    nc = bacc.Bacc("TRN2", debug=False, num_devices=8)

    q_d = nc.dram_tensor("q", [L, E], F32, kind="ExternalInput")
    kvt_d = nc.dram_tensor("kvt", [ET * P, LT * P], R, kind="ExternalInput")
    slab_d = {}
    for nm in ("ce", "se", "co", "so"):
        slab_d[nm] = nc.dram_tensor(nm, [P, LH * 513], R, kind="ExternalInput")
    ig_d = {}
    for nm in ("iec", "ies", "ioc", "ios"):
        ig_d[nm] = nc.dram_tensor(nm, [513, 1024], R, kind="ExternalInput")
    W_d = {}
    for nm in ("qr", "qi", "kr", "ki", "vr", "vi"):
        W_d[nm] = nc.dram_tensor(f"W{nm}", [E, 512], R, kind="ExternalInput")
        W_d["b" + nm] = nc.dram_tensor(f"b{nm}", [512, 1], F32, kind="ExternalInput")
    dc_d = {nm: nc.dram_tensor(f"dc{nm}", [512, 1], F32, kind="ExternalInput")
            for nm in ("qr", "qi")}
    WoT_d = nc.dram_tensor("WoT", [512, E], R, kind="ExternalInput")
    out_d = nc.dram_tensor("out", [L, E], F32, kind="ExternalOutput")

    with tile.TileContext(nc) as tc:
        with tc.tile_pool(name="dram", bufs=1, space="DRAM") as dram, \
             tc.tile_pool(name="persist", bufs=1) as persist, \
             tc.tile_pool(name="kcl", bufs=1) as kcl, \
             tc.tile_pool(name="vcl", bufs=1) as vcl, \
             tc.tile_pool(name="oacc", bufs=1) as oacc:
            Qcat_dram = dram.tile([NH, P, FP], R)

            eps_t = persist.tile([P, 1], F32)
            nc.vector.memset(eps_t[:], EPS)

            # K and V stay SBUF-resident from projection through attention
            Kc = [kcl.tile([P, FP], R, tag=f"Kc{h}", name=f"Kc{h}")
                  for h in range(NH)]
            Vc = [vcl.tile([P, NH * 129], BF16, tag=f"Vc{ti}", name=f"Vc{ti}")
                  for ti in range(len(MTI))]

            our = []
            oui = []
            for ti in range(len(MTI)):
                our.append(oacc.tile([P, 512], R, tag=f"our{ti}", name=f"our{ti}"))
                oui.append(oacc.tile([P, 512], R, tag=f"oui{ti}", name=f"oui{ti}"))

            # ---------- radix-2 DFT + projection worker ----------
            def dft_proj(is_q, get_xe, Wt, bias_t, vbias, slabs, dps, qfp, stg, dct):
                pnames = ("qr", "qi") if is_q else ("kr", "ki")
                for jc, (j0, jsz) in enumerate(JCH):
                    bw = 256   # B-block width in this chunk
                    xa_r, xa_i, xb_r, xb_i = [], [], [], []
                    for eb in range(ET):
                        xe = get_xe(eb)
                        pEr = dps.tile([P, jsz], F32, tag="pEr", name="pEr")
                        pEi = dps.tile([P, jsz], F32, tag="pEi", name="pEi")
                        pPr = dps.tile([P, jsz], F32, tag="pPr", name="pPr")
                        pPi = dps.tile([P, jsz], F32, tag="pPi", name="pPi")
                        for lh in range(LH):
                            sl = slice(lh * jsz, (lh + 1) * jsz)
                            st_, sp_ = (lh == 0), (lh == LH - 1)
                            nc.tensor.matmul(pEr[:], xe(lh), slabs["ce"][jc][:, sl],
                                             start=st_, stop=sp_)
                            nc.tensor.matmul(pEi[:], xe(lh), slabs["se"][jc][:, sl],
                                             start=st_, stop=sp_)
                            nc.tensor.matmul(pPr[:], xe(LH + lh), slabs["co"][jc][:, sl],
                                             start=st_, stop=sp_)
                            nc.tensor.matmul(pPi[:], xe(LH + lh), slabs["so"][jc][:, sl],
                                             start=st_, stop=sp_)
                        # E -> SBUF via ACT, then 4 DVE combines (SBUF, PSUM)
                        erS = stg.tile([P, 257], F32, tag="erS", name="erS", bufs=2)
                        eiS = stg.tile([P, 257], F32, tag="eiS", name="eiS", bufs=2)
                        nc.scalar.activation(erS[:, 0:jsz], pEr[:], AF.Copy)
                        nc.scalar.activation(eiS[:, 0:jsz], pEi[:], AF.Copy)
                        ar = qfp.tile([P, jsz], R, tag=f"ar{eb}", name=f"ar{eb}")
                        ai = qfp.tile([P, jsz], R, tag=f"ai{eb}", name=f"ai{eb}")
                        br = qfp.tile([P, bw], R, tag=f"br{eb}", name=f"br{eb}")
                        bi = qfp.tile([P, bw], R, tag=f"bi{eb}", name=f"bi{eb}")
                        nc.vector.tensor_add(ar[:], erS[:, 0:jsz], pPr[:])
                        nc.vector.tensor_add(ai[:], eiS[:, 0:jsz], pPi[:])
                        nc.vector.tensor_sub(br[:], erS[:, 0:bw], pPr[:, 0:bw])
                        nc.vector.tensor_sub(bi[:], pPi[:, 0:bw], eiS[:, 0:bw])
                        xa_r.append(ar)
                        xa_i.append(ai)
                        xb_r.append(br)
                        xb_i.append(bi)

                    for (u0, w, jcc, kind) in SCH:
                        if jcc != jc:
                            continue
                        xr = xa_r if kind == 'A' else xb_r
                        xi = xa_i if kind == 'A' else xb_i
                        # Q/K projections for this storage chunk
                        for mt in range(4):
                            pps = {}
                            for nm in pnames:
                                pps[nm] = dps.tile([P, w], F32, tag=f"pp{nm}", name=f"pp{nm}")
                            for ec in range(ET):
                                src = {pnames[0]: xr[ec], pnames[1]: xi[ec]}
                                for nm in pnames:
                                    nc.tensor.matmul(pps[nm][:],
                                                     Wt[nm][ec][:, mt * P:(mt + 1) * P],
                                                     src[nm][:],
                                                     start=(ec == 0), stop=(ec == ET - 1))
                            sg = {}
                            for nm in pnames:
                                s = stg.tile([P, 257], R, tag=f"sg{nm}", name=f"sg{nm}", bufs=2)
                                nc.scalar.activation(s[:, 0:w], pps[nm][:], AF.Identity,
                                                     bias=bias_t[nm][mt][:])
                                if is_q and u0 == 0:
                                    # beta's DC contribution (storage col u=0)
                                    nc.vector.scalar_tensor_tensor(
                                        s[:, 0:1], dct[nm][mt][:], 1.0,
                                        s[:, 0:1],
                                        op0=mybir.AluOpType.mult,
                                        op1=mybir.AluOpType.add)
                                sg[nm] = s
                            r0, i0 = pnames
                            if is_q:
                                nc.sync.dma_start(Qcat_dram[2 * mt, 0:64, u0:u0 + w], sg[r0][0:64, 0:w])
                                nc.sync.dma_start(Qcat_dram[2 * mt + 1, 0:64, u0:u0 + w], sg[r0][64:128, 0:w])
                                nc.sync.dma_start(Qcat_dram[2 * mt, 64:128, u0:u0 + w], sg[i0][0:64, 0:w])
                                nc.sync.dma_start(Qcat_dram[2 * mt + 1, 64:128, u0:u0 + w], sg[i0][64:128, 0:w])
                            else:
                                # SBUF->SBUF partition-shuffling DMAs into Kc
                                nc.sync.dma_start(Kc[2 * mt][0:64, u0:u0 + w], sg[r0][0:64, 0:w])
                                nc.sync.dma_start(Kc[2 * mt + 1][0:64, u0:u0 + w], sg[r0][64:128, 0:w])
                                nc.sync.dma_start(Kc[2 * mt][64:128, u0:u0 + w], sg[i0][0:64, 0:w])
                                nc.sync.dma_start(Kc[2 * mt + 1][64:128, u0:u0 + w], sg[i0][64:128, 0:w])

                        # V projection (kv path only) -> directly into Vc
                        if not is_q:
                            for vti, (m0, msz) in enumerate(MTI):
                                if not (m0 >= u0 and m0 + msz <= u0 + w):
                                    continue
                                mr = m0 - u0
                                pvr = dps.tile([P, 512], F32, tag="pvr", name="pvr")
                                pvi = dps.tile([P, 512], F32, tag="pvi", name="pvi")
                                for ec in range(ET):
                                    nc.tensor.matmul(pvr[0:msz, :], xr[ec][:, mr:mr + msz],
                                                     Wt["vr"][ec][:],
                                                     start=(ec == 0), stop=(ec == ET - 1))
                                    nc.tensor.matmul(pvi[0:msz, :], xi[ec][:, mr:mr + msz],
                                                     Wt["vi"][ec][:],
                                                     start=(ec == 0), stop=(ec == ET - 1))
                                vco = Vc[vti][0:msz, :].rearrange("p (h c) -> p h c", h=NH)
                                nc.vector.tensor_add(
                                    vco[:, :, 0:64],
                                    pvr[0:msz, :].rearrange("p (h c) -> p h c", h=NH),
                                    vbias["vr"][0:msz, :].rearrange("p (h c) -> p h c", h=NH))
                                nc.vector.tensor_add(
                                    vco[:, :, 64:128],
                                    pvi[0:msz, :].rearrange("p (h c) -> p h c", h=NH),
                                    vbias["vi"][0:msz, :].rearrange("p (h c) -> p h c", h=NH))
                                nc.vector.memset(vco[:, :, 128:129], 1.0)

            # ================= Phase 1 =================
            with tc.tile_pool(name="qn", bufs=1) as qnp, \
                 tc.tile_pool(name="fsl", bufs=1) as fslp, \
                 tc.tile_pool(name="qf", bufs=1) as qfp, \
                 tc.tile_pool(name="stg", bufs=2) as stg:
                # DFT basis slabs (shared by kv and q paths), loaded once,
                # split per j-chunk so only jc0 halves gate the first MMs
                slabs = {nm: [None, None] for nm in ("ce", "se", "co", "so")}

                def load_slabs(jc):
                    j0, jsz = JCH[jc]
                    for nm in ("ce", "se", "co", "so"):
                        t = fslp.tile([P, LH * jsz], R, tag=f"{nm}{jc}", name=f"{nm}{jc}")
                        nc.sync.dma_start(
                            t[:], slab_d[nm].ap()[:, JOFF[jc]:JOFF[jc] + LH * jsz])
                        slabs[nm][jc] = t

                qn_t = [qnp.tile([P, E], R, tag=f"qn{lc}", name=f"qn{lc}")
                        for lc in range(LT)]

                # ---- Phase 1a: kv path (kv tiles streamed per (jc, eb)) ----
                with tc.tile_pool(name="kvs", bufs=1) as kvp, \
                     tc.tile_pool(name="wkv", bufs=1) as wkv, \
                     tc.tile_pool(name="dpsa", bufs=1, space="PSUM") as dpsa:
                    def load_kvt(eb):
                        t = kvp.tile([P, LT * P], R, tag=f"kvs{eb % 3}",
                                     name=f"kvs{eb % 3}")
                        nc.sync.dma_start(t[:], kvt_d.ap()[eb * P:(eb + 1) * P, :])
                        return t
                    # startup-critical DMA order: kvt0, jc0 slabs, kvt1-2,
                    # jc1 slabs, then weights
                    kv_pre = {0: load_kvt(0)}
                    load_slabs(0)
                    kv_pre[1] = load_kvt(1)
                    kv_pre[2] = load_kvt(2)
                    load_slabs(1)

                    def get_xe_kv(eb):
                        t = kv_pre.pop(eb) if eb in kv_pre else load_kvt(eb)
                        return lambda lc: t[:, lc * P:(lc + 1) * P]
                    Wt = {}
                    bias_t = {}
                    for nm in ("kr", "ki", "vr", "vi"):
                        Wt[nm] = [wkv.tile([P, 512], R, tag=f"W{nm}{ec}", name=f"W{nm}{ec}")
                                  for ec in range(ET)]
                        for ec in range(ET):
                            nc.scalar.dma_start(Wt[nm][ec][:],
                                                W_d[nm].ap()[ec * P:(ec + 1) * P, :])
                        if nm in ("kr", "ki"):
                            bias_t[nm] = [wkv.tile([P, 1], F32, tag=f"b{nm}{mt}", name=f"b{nm}{mt}")
                                          for mt in range(4)]
                            for mt in range(4):
                                nc.scalar.dma_start(bias_t[nm][mt][:],
                                                    W_d["b" + nm].ap()[mt * P:(mt + 1) * P, :])
                    vb_row = wkv.tile([1, 512], F32, tag="vbrow", name="vbrow")
                    vbias = {}
                    for nm in ("vr", "vi"):
                        nc.scalar.dma_start(vb_row[:], W_d["b" + nm].ap().rearrange("e one -> one e"))
                        vb = wkv.tile([P, 512], F32, tag=f"vb{nm}", name=f"vb{nm}")
                        nc.gpsimd.partition_broadcast(vb[:], vb_row[:])
                        vbias[nm] = vb

                    dft_proj(False, get_xe_kv,
                             Wt, bias_t, vbias, slabs, dpsa, qfp, stg, None)

                    # LN of q (emitted after kv work: q DMAs queue behind
                    # kv-phase loads; DVE/ACT fill in around kv evictions)
                    with tc.tile_pool(name="ln", bufs=2) as ln, \
                         tc.tile_pool(name="lns", bufs=4) as lns:
                        for lc in range(LT):
                            qt = ln.tile([P, E], F32, tag="qt", name="qt")
                            nc.sync.dma_start(qt[:], q_d.ap()[lc * P:(lc + 1) * P, :])
                            st = lns.tile([P, 12], F32, tag="st", name="st")
                            nc.vector.bn_stats(st[:, 0:6], qt[:, 0:512])
                            nc.vector.bn_stats(st[:, 6:12], qt[:, 512:1024])
                            mv = lns.tile([P, 2], F32, tag="mv", name="mv")
                            nc.vector.bn_aggr(mv[:], st[:])
                            sd = lns.tile([P, 1], F32, tag="sd", name="sd")
                            nc.scalar.activation(sd[:], mv[:, 1:2], AF.Sqrt, bias=eps_t[:])
                            istd = lns.tile([P, 1], F32, tag="istd", name="istd")
                            nc.vector.reciprocal(istd[:], sd[:])
                            nmu = lns.tile([P, 1], F32, tag="nmu", name="nmu")
                            nc.vector.tensor_scalar_mul(nmu[:], mv[:, 0:1], -1.0)
                            nc.vector.tensor_mul(nmu[:], nmu[:], istd[:])
                            nc.scalar.activation(qn_t[lc][:], qt[:], AF.Identity,
                                                 bias=nmu[:], scale=istd[:])

                # ---- Phase 1b: q path ----
                with tc.tile_pool(name="wq", bufs=1) as wq, \
                     tc.tile_pool(name="dpsb", bufs=1, space="PSUM") as dpsb:
                    Wtq = {}
                    bias_q = {}
                    dct = {}
                    for nm in ("qr", "qi"):
                        Wtq[nm] = [wq.tile([P, 512], R, tag=f"W{nm}{ec}", name=f"W{nm}{ec}")
                                   for ec in range(ET)]
                        for ec in range(ET):
                            nc.sync.dma_start(Wtq[nm][ec][:],
                                                W_d[nm].ap()[ec * P:(ec + 1) * P, :])
                        bias_q[nm] = [wq.tile([P, 1], F32, tag=f"b{nm}{mt}", name=f"b{nm}{mt}")
                                      for mt in range(4)]
                        dct[nm] = [wq.tile([P, 1], F32, tag=f"dc{nm}{mt}", name=f"dc{nm}{mt}")
                                   for mt in range(4)]
                        for mt in range(4):
                            nc.sync.dma_start(bias_q[nm][mt][:],
                                                W_d["b" + nm].ap()[mt * P:(mt + 1) * P, :])
                            nc.sync.dma_start(dct[nm][mt][:],
                                                dc_d[nm].ap()[mt * P:(mt + 1) * P, :])
                    dft_proj(True,
                             lambda eb: (lambda lc: qn_t[lc][:, eb * P:(eb + 1) * P]),
                             Wtq, bias_q, None, slabs, dpsb, qfp, stg, dct)

            # ================= Phase 2: attention =================
            attn_ctx = [tc.tile_pool(name="qk", bufs=1),
                        tc.tile_pool(name="expp", bufs=3),
                        tc.tile_pool(name="sps", bufs=4, space="PSUM"),
                        tc.tile_pool(name="avps", bufs=3, space="PSUM"),
                        tc.tile_pool(name="nrm", bufs=4)]
            qk, expp, sps, avps, nrm = [c.__enter__() for c in attn_ctx]
            Qc = []
            for h in range(NH):
                qt = qk.tile([P, FP], R, tag=f"Qc{h}", name=f"Qc{h}")
                nc.sync.dma_start(qt[:], Qcat_dram[h, :, :])
                Qc.append(qt)

            def do_av(h, expts):
                for ti, (l0, lsz) in enumerate(MTI):
                    ps = avps.tile([P, 129], F32, tag="av", name="av")
                    n = len(MTI)
                    for mi, (m0, msz) in enumerate(MTI):
                        nc.tensor.matmul(ps[0:lsz, :], expts[mi][0:msz, l0:l0 + lsz],
                                         Vc[mi][0:msz, h * 129:(h + 1) * 129],
                                         start=(mi == 0), stop=(mi == n - 1))
                    rcp = nrm.tile([P, 1], F32, tag="rcp", name="rcp")
                    nc.vector.reciprocal(rcp[0:lsz, :], ps[0:lsz, 128:129])
                    nc.vector.tensor_scalar_mul(our[ti][0:lsz, h * 64:(h + 1) * 64],
                                                ps[0:lsz, 0:64], rcp[0:lsz, :])
                    nc.vector.tensor_scalar_mul(oui[ti][0:lsz, h * 64:(h + 1) * 64],
                                                ps[0:lsz, 64:128], rcp[0:lsz, :])

            # software pipeline: AV for head h-1 overlaps scores/exp for h
            prev = None
            for h in range(NH):
                expts = []
                for ti, (m0, msz) in enumerate(MTI):
                    et_ = expp.tile([P, FP], BF16, tag=f"exp{ti}", name=f"exp{ti}")
                    for (f0, fsz) in FCH:
                        ps = sps.tile([P, 384], F32, tag="sc", name="sc")
                        nc.tensor.matmul(ps[0:msz, 0:fsz], Kc[h][:, m0:m0 + msz],
                                         Qc[h][:, f0:f0 + fsz], start=True, stop=True)
                        nc.scalar.activation(et_[0:msz, f0:f0 + fsz], ps[0:msz, 0:fsz],
                                             AF.Exp, scale=float(D ** -0.5))
                    expts.append(et_)
                if prev is not None:
                    do_av(h - 1, prev)
                prev = expts
            do_av(NH - 1, prev)
            for c in reversed(attn_ctx):
                c.__exit__(None, None, None)

            # ================= Phase 3: radix-2 iDFT + Wo =================
            # OTT columns hold [even times tau 0..1023 | odd times]; the
            # final output DMA de-interleaves via a strided DRAM view.
            with tc.tile_pool(name="gsl", bufs=3) as gsl, \
                 tc.tile_pool(name="uvp", bufs=1) as uvp, \
                 tc.tile_pool(name="ott", bufs=1) as ottp, \
                 tc.tile_pool(name="wop", bufs=1) as wop, \
                 tc.tile_pool(name="ost", bufs=3) as ost:
                OTT = [ottp.tile([P, L], R, tag=f"OTT{i}", name=f"OTT{i}") for i in range(4)]
                WoT_t = [wop.tile([P, E], R, tag=f"wo{i}", name=f"wo{i}") for i in range(4)]
                for ec in range(4):
                    nc.sync.dma_start(WoT_t[ec][:], WoT_d.ap()[ec * P:(ec + 1) * P, :])
                # pre-halve DC (u=0) and Nyquist (u=513 -> k=1024) rows;
                # basis matrices carry the x2 weighting for all rows
                for t_ in (our[0], oui[0], our[5], oui[5]):
                    nc.vector.tensor_scalar_mul(t_[0:1, :], t_[0:1, :], 0.5)
                Ur, Ui, Vr, Vi = [], [], [], []
                for jt in range(4):
                    u_r = uvp.tile([P, 512], R, tag=f"Ur{jt}", name=f"Ur{jt}")
                    u_i = uvp.tile([P, 512], R, tag=f"Ui{jt}", name=f"Ui{jt}")
                    v_r = uvp.tile([P, 512], R, tag=f"Vr{jt}", name=f"Vr{jt}")
                    v_i = uvp.tile([P, 512], R, tag=f"Vi{jt}", name=f"Vi{jt}")
                    nc.vector.tensor_add(u_r[:], our[jt][:], our[5 + jt][:])
                    nc.vector.tensor_sub(v_r[:], our[jt][:], our[5 + jt][:])
                    nc.vector.tensor_sub(u_i[:], oui[jt][:], oui[5 + jt][:])
                    nc.vector.tensor_add(v_i[:], oui[jt][:], oui[5 + jt][:])
                    Ur.append(u_r)
                    Ui.append(u_i)
                    Vr.append(v_r)
                    Vi.append(v_i)

                out_v = out_d.ap().rearrange("(t two) e -> two t e", two=2)
                idps_ctx = tc.tile_pool(name="idps", bufs=1, space="PSUM")
                idps = idps_ctx.__enter__()
                wops_ctx = tc.tile_pool(name="wops", bufs=2, space="PSUM")
                wops = wops_ctx.__enter__()

                def wo_block(tb):
                    pso = [wops.tile([P, 512], F32, tag=f"po{eo}", name=f"po{eo}")
                           for eo in range(2)]
                    for eo in range(2):
                        for ec in range(4):
                            nc.tensor.matmul(pso[eo][:],
                                             OTT[ec][:, tb * P:(tb + 1) * P],
                                             WoT_t[ec][:, eo * 512:(eo + 1) * 512],
                                             start=(ec == 0), stop=(ec == 3))
                    ot_ = ost.tile([P, E], F32, tag="ot", name="ot")
                    nc.vector.tensor_copy(ot_[:, 0:512], pso[0][:])
                    nc.scalar.activation(ot_[:, 512:1024], pso[1][:], AF.Copy)
                    nc.sync.dma_start(
                        out_v[tb // 8, (tb % 8) * P:(tb % 8 + 1) * P, :], ot_[:])

                # 4-bank iDFT passes (oh x e4-pair) so Wo for the even half
                # overlaps the odd-half iDFT on the PE
                for oh in range(2):
                    nm_c, nm_s = ("iec", "ies") if oh == 0 else ("ioc", "ios")
                    for ep in range(2):
                        pst = [[idps.tile([P, 512], F32, tag=f"ph{i}_{t2}", name=f"ph{i}_{t2}")
                                for t2 in range(2)] for i in range(2)]
                        for jt in range(5):
                            msz = P if jt < 4 else 1
                            r0 = jt * P
                            mc = gsl.tile([P, 1024], R, tag="gc", name="gc")
                            ms = gsl.tile([P, 1024], R, tag="gs", name="gs")
                            nc.sync.dma_start(mc[0:msz, :], ig_d[nm_c].ap()[r0:r0 + msz, :])
                            nc.sync.dma_start(ms[0:msz, :], ig_d[nm_s].ap()[r0:r0 + msz, :])
                            if jt < 4:
                                sr = Ur[jt] if oh == 0 else Vr[jt]
                                si = Ui[jt] if oh == 0 else Vi[jt]
                            else:
                                sr, si = our[4], oui[4]
                            for i in range(2):
                                e4 = 2 * ep + i
                                for t2 in range(2):
                                    nc.tensor.matmul(pst[i][t2][:],
                                                     sr[0:msz, e4 * P:(e4 + 1) * P],
                                                     mc[0:msz, t2 * 512:(t2 + 1) * 512],
                                                     start=(jt == 0), stop=False)
                                    nc.tensor.matmul(pst[i][t2][:],
                                                     si[0:msz, e4 * P:(e4 + 1) * P],
                                                     ms[0:msz, t2 * 512:(t2 + 1) * 512],
                                                     start=False, stop=(jt == 4))
                        for i in range(2):
                            e4 = 2 * ep + i
                            for t2 in range(2):
                                dst = OTT[e4][:, oh * 1024 + t2 * 512:oh * 1024 + (t2 + 1) * 512]
                                if i == 0:
                                    nc.vector.tensor_copy(dst, pst[i][t2][:])
                                else:
                                    nc.scalar.activation(dst, pst[i][t2][:], AF.Copy)
                    if oh == 0:
                        for tb in range(8):
                            wo_block(tb)
                for tb in range(8, LT):
                    wo_block(tb)
                wops_ctx.__exit__(None, None, None)
                idps_ctx.__exit__(None, None, None)

    nc.finalize()
    return nc


def kernel(**inputs):
    from concourse.bass_utils import run_bass_kernel_spmd

    rdt = ml_dtypes.bfloat16 if MM_BF16 else np.float32
    if "nc" not in _CACHE:
        _CACHE["nc"] = _build()
        Ce, Se, Co, So, Ec, Es, Oc, Os = _dft_consts()
        _CACHE["consts"] = {
            "ce": _pretile_half(Ce).astype(rdt),
            "se": _pretile_half(Se).astype(rdt),
            "co": _pretile_half(Co).astype(rdt),
            "so": _pretile_half(So).astype(rdt),
            "iec": Ec.astype(rdt), "ies": Es.astype(rdt),
            "ioc": Oc.astype(rdt), "ios": Os.astype(rdt),
        }
    nc = _CACHE["nc"]
    C = _CACHE["consts"]

    perm = np.r_[0:L:2, 1:L:2]
    q = np.ascontiguousarray(inputs["query"], dtype=np.float32)[:, perm, :]
    kv = np.asarray(inputs["key_value"], dtype=np.float32)[:, perm, :]
    gamma = np.asarray(inputs["gamma"], np.float32)
    beta = np.asarray(inputs["beta"], np.float32)
    in_maps = []
    for core in range(8):
        b = core // 2
        hg = core % 2
        cs = slice(hg * 512, (hg + 1) * 512)
        kvt = np.ascontiguousarray(
            kv[b].reshape(LT, P, ET, P).transpose(2, 1, 0, 3).reshape(ET * P, LT * P)
        ).astype(rdt)
        m = {
            "q": np.ascontiguousarray(q[b]),
            "kvt": kvt,
            "ce": C["ce"], "se": C["se"], "co": C["co"], "so": C["so"],
            "iec": C["iec"], "ies": C["ies"], "ioc": C["ioc"], "ios": C["ios"],
            "WoT": np.ascontiguousarray(inputs["Wo"][:, cs].T.astype(rdt)),
        }
        for nm in ("qr", "qi", "kr", "ki", "vr", "vi"):
            Wcs = np.asarray(inputs["W" + nm], np.float32)[cs, :]
            if nm in ("qr", "qi"):
                m[f"dc{nm}"] = np.ascontiguousarray(
                    (Wcs @ beta) * math.sqrt(L), np.float32).reshape(512, 1)
                Wcs = Wcs * gamma[None, :]
            m[f"W{nm}"] = np.ascontiguousarray(Wcs.T.astype(rdt))
            m[f"b{nm}"] = np.ascontiguousarray(inputs["b" + nm][cs], np.float32).reshape(512, 1)
        in_maps.append(m)

    res = run_bass_kernel_spmd(nc, in_maps, core_ids=list(range(8)))
    _CACHE["last"] = res
    out = np.empty((B, L, E), np.float32)
    for b in range(B):
        out[b] = res.results[2 * b]["out"] + res.results[2 * b + 1]["out"]
    return out


# revision 41
# speedup vs baseline: 1.0559x; 1.0299x over previous
"""Trainium2 Bass kernel for nn_FreqCrossAttention.

Sharding: 8 cores = 4 batches x 2 head-groups (8 heads each).
Each core computes a partial output [2048, 1024] (its head-group's
contribution through W_o row-parallel); host sums the pair per batch.

v3: radix-2 DFT. Host permutes the L axis into (even, odd) halves; the
rfft becomes two 1024-point half-DFTs (twiddles folded into the odd
basis matrices) plus a 4-op combine:
    A (freqs k=0..512)        = E + P
    B (freqs k=1024..513)     : Br = Er - Pr, Bi = Pi - Ei
Frequencies are stored in "split order" [A | B]; the inverse
permutation is folded into the host-built iDFT matrices. Gamma is
folded into Wqr/Wqi host-side; beta's DC term is added on-device to
storage column u=0 post-projection. DFT matmul FLOPs halve.
"""
import math
import numpy as np
import ml_dtypes

MM_BF16 = True

B, L, E, H = 4, 2048, 1024, 16
D = E // H            # 64
Lf = L // 2 + 1       # 1025
FP = 1026             # padded frequency dim (split-order + 1 pad col)
NH = 8                # heads per core
P = 128
LH = 8                # L-tiles per half (even/odd)
JCH = [(0, 256), (256, 257)]          # j-chunks of the 513 half-freqs
JOFF = [0, LH * 256]                  # free offsets in pretiled slabs
# storage chunks: (u0, width, j-chunk index, block)
SCH = [(0, 256, 0, 'A'), (256, 257, 1, 'A'),
       (513, 256, 0, 'B'), (769, 256, 1, 'B')]
# m-tiles in storage order (1025 real bins; singleton at u=512);
# also used as l-tiles for AV and j-tiles for the radix-2 iDFT
MTI = [(0, 128), (128, 128), (256, 128), (384, 128), (512, 1),
       (513, 128), (641, 128), (769, 128), (897, 128)]
FCH = [(0, 384), (384, 384), (768, 257)]   # scores moving-dim chunks
ET = 8                # e-chunks of E
LT = 16               # L tiles
EPS = 1e-5

_CACHE = {}


def _pretile_half(M):
    # [1024, 513] -> [P, LH*513] chunk-major (j-chunks of JCH)
    r = M.reshape(LH, P, 513)
    blocks = []
    for (j0, jsz) in JCH:
        blocks.append(np.ascontiguousarray(
            r[:, :, j0:j0 + jsz].transpose(1, 0, 2).reshape(P, LH * jsz)))
    return np.concatenate(blocks, axis=1)              # [P, LH*513]


def _dft_consts():
    s = 1.0 / math.sqrt(L)
    m = np.arange(1024)[:, None].astype(np.float64)
    j = np.arange(513)[None, :].astype(np.float64)
    ang_e = 2.0 * np.pi * (2.0 * m) * j / L
    ang_o = 2.0 * np.pi * (2.0 * m + 1.0) * j / L
    Ce = (np.cos(ang_e) * s).astype(np.float32)
    Se = (-np.sin(ang_e) * s).astype(np.float32)
    Co = (np.cos(ang_o) * s).astype(np.float32)
    So = (-np.sin(ang_o) * s).astype(np.float32)
    # radix-2 iDFT half-matrices (cw doubling pre-folded: rows x2; the
    # DC/Nyquist rows of our/oui get pre-halved on device instead)
    tau = np.arange(1024)[None, :].astype(np.float64)
    jj = np.arange(513)[:, None].astype(np.float64)
    Ec = (2.0 * s * np.cos(2.0 * np.pi * jj * tau / 1024.0)).astype(np.float32)
    Es = (-2.0 * s * np.sin(2.0 * np.pi * jj * tau / 1024.0)).astype(np.float32)
    Oc = (2.0 * s * np.cos(np.pi * jj * (2.0 * tau + 1.0) / 1024.0)).astype(np.float32)
    Os = (-2.0 * s * np.sin(np.pi * jj * (2.0 * tau + 1.0) / 1024.0)).astype(np.float32)
    return Ce, Se, Co, So, Ec, Es, Oc, Os


def _build():
    import concourse.bass as bass
    import concourse.bacc as bacc
    import concourse.mybir as mybir
    import concourse.tile as tile

    R = mybir.dt.bfloat16 if MM_BF16 else mybir.dt.float32r
    F32 = mybir.dt.float32
    BF16 = mybir.dt.bfloat16
    AF = mybir.ActivationFunctionType

    nc = bacc.Bacc("TRN2", debug=False, num_devices=8)

    q_d = nc.dram_tensor("q", [L, E], F32, kind="ExternalInput")
    kvt_d = nc.dram_tensor("kvt", [ET * P, LT * P], R, kind="ExternalInput")
    slab_d = {}
    for nm in ("ce", "se", "co", "so"):
        slab_d[nm] = nc.dram_tensor(nm, [P, LH * 513], R, kind="ExternalInput")
    ig_d = {}
    for nm in ("iec", "ies", "ioc", "ios"):
        ig_d[nm] = nc.dram_tensor(nm, [513, 1024], R, kind="ExternalInput")
    W_d = {}
    for nm in ("qr", "qi", "kr", "ki", "vr", "vi"):
        W_d[nm] = nc.dram_tensor(f"W{nm}", [E, 512], R, kind="ExternalInput")
        W_d["b" + nm] = nc.dram_tensor(f"b{nm}", [512, 1], F32, kind="ExternalInput")
    dc_d = {nm: nc.dram_tensor(f"dc{nm}", [512, 1], F32, kind="ExternalInput")
            for nm in ("qr", "qi")}
    WoT_d = nc.dram_tensor("WoT", [512, E], R, kind="ExternalInput")
    out_d = nc.dram_tensor("out", [L, E], F32, kind="ExternalOutput")

    with tile.TileContext(nc) as tc:
        with tc.tile_pool(name="dram", bufs=1, space="DRAM") as dram, \
             tc.tile_pool(name="persist", bufs=1) as persist, \
             tc.tile_pool(name="kcl", bufs=1) as kcl, \
             tc.tile_pool(name="vcl", bufs=1) as vcl, \
             tc.tile_pool(name="oacc", bufs=1) as oacc:
            qcat_t = [dram.tile([P, FP], R, tag=f"qcat{h}", name=f"qcat{h}")
                      for h in range(NH)]

            eps_t = persist.tile([P, 1], F32)
            nc.vector.memset(eps_t[:], EPS)

            # K and V stay SBUF-resident from projection through attention
            Kc = [kcl.tile([P, FP], R, tag=f"Kc{h}", name=f"Kc{h}")
                  for h in range(NH)]
            Vc = [vcl.tile([P, NH * 129], BF16, tag=f"Vc{ti}", name=f"Vc{ti}")
                  for ti in range(len(MTI))]

            our = []
            oui = []
            for ti in range(len(MTI)):
                our.append(oacc.tile([P, 512], R, tag=f"our{ti}", name=f"our{ti}"))
                oui.append(oacc.tile([P, 512], R, tag=f"oui{ti}", name=f"oui{ti}"))

            # ---------- radix-2 DFT + projection worker ----------
            def dft_proj(is_q, get_xe, Wt, bias_t, vbias, slabs, dps, qfp, stg, dct):
                pnames = ("qr", "qi") if is_q else ("kr", "ki")
                for jc, (j0, jsz) in enumerate(JCH):
                    bw = 256   # B-block width in this chunk
                    xa_r, xa_i, xb_r, xb_i = [], [], [], []
                    for eb in range(ET):
                        xe = get_xe(eb)
                        pEr = dps.tile([P, jsz], F32, tag="pEr", name="pEr")
                        pEi = dps.tile([P, jsz], F32, tag="pEi", name="pEi")
                        pPr = dps.tile([P, jsz], F32, tag="pPr", name="pPr")
                        pPi = dps.tile([P, jsz], F32, tag="pPi", name="pPi")
                        for lh in range(LH):
                            sl = slice(lh * jsz, (lh + 1) * jsz)
                            st_, sp_ = (lh == 0), (lh == LH - 1)
                            nc.tensor.matmul(pEr[:], xe(lh), slabs["ce"][jc][:, sl],
                                             start=st_, stop=sp_)
                            nc.tensor.matmul(pEi[:], xe(lh), slabs["se"][jc][:, sl],
                                             start=st_, stop=sp_)
                            nc.tensor.matmul(pPr[:], xe(LH + lh), slabs["co"][jc][:, sl],
                                             start=st_, stop=sp_)
                            nc.tensor.matmul(pPi[:], xe(LH + lh), slabs["so"][jc][:, sl],
                                             start=st_, stop=sp_)
                        # E -> SBUF via ACT, then 4 DVE combines (SBUF, PSUM)
                        erS = stg.tile([P, 257], F32, tag="erS", name="erS", bufs=2)
                        eiS = stg.tile([P, 257], F32, tag="eiS", name="eiS", bufs=2)
                        nc.scalar.activation(erS[:, 0:jsz], pEr[:], AF.Copy)
                        nc.scalar.activation(eiS[:, 0:jsz], pEi[:], AF.Copy)
                        ar = qfp.tile([P, jsz], R, tag=f"ar{eb}", name=f"ar{eb}")
                        ai = qfp.tile([P, jsz], R, tag=f"ai{eb}", name=f"ai{eb}")
                        br = qfp.tile([P, bw], R, tag=f"br{eb}", name=f"br{eb}")
                        bi = qfp.tile([P, bw], R, tag=f"bi{eb}", name=f"bi{eb}")
                        nc.vector.tensor_add(ar[:], erS[:, 0:jsz], pPr[:])
                        nc.vector.tensor_add(ai[:], eiS[:, 0:jsz], pPi[:])
                        nc.vector.tensor_sub(br[:], erS[:, 0:bw], pPr[:, 0:bw])
                        nc.vector.tensor_sub(bi[:], pPi[:, 0:bw], eiS[:, 0:bw])
                        xa_r.append(ar)
                        xa_i.append(ai)
                        xb_r.append(br)
                        xb_i.append(bi)

                    for (u0, w, jcc, kind) in SCH:
                        if jcc != jc:
                            continue
                        xr = xa_r if kind == 'A' else xb_r
                        xi = xa_i if kind == 'A' else xb_i
                        # Q/K projections for this storage chunk
                        for mt in range(4):
                            pps = {}
                            for nm in pnames:
                                pps[nm] = dps.tile([P, w], F32, tag=f"pp{nm}", name=f"pp{nm}")
                            for ec in range(ET):
                                src = {pnames[0]: xr[ec], pnames[1]: xi[ec]}
                                for nm in pnames:
                                    nc.tensor.matmul(pps[nm][:],
                                                     Wt[nm][ec][:, mt * P:(mt + 1) * P],
                                                     src[nm][:],
                                                     start=(ec == 0), stop=(ec == ET - 1))
                            sg = {}
                            for nm in pnames:
                                s = stg.tile([P, 257], R, tag=f"sg{nm}", name=f"sg{nm}", bufs=2)
                                nc.scalar.activation(s[:, 0:w], pps[nm][:], AF.Identity,
                                                     bias=bias_t[nm][mt][:])
                                if is_q and u0 == 0:
                                    # beta's DC contribution (storage col u=0)
                                    nc.vector.scalar_tensor_tensor(
                                        s[:, 0:1], dct[nm][mt][:], 1.0,
                                        s[:, 0:1],
                                        op0=mybir.AluOpType.mult,
                                        op1=mybir.AluOpType.add)
                                sg[nm] = s
                            r0, i0 = pnames
                            if is_q:
                                nc.sync.dma_start(qcat_t[2 * mt][0:64, u0:u0 + w], sg[r0][0:64, 0:w])
                                nc.sync.dma_start(qcat_t[2 * mt + 1][0:64, u0:u0 + w], sg[r0][64:128, 0:w])
                                nc.sync.dma_start(qcat_t[2 * mt][64:128, u0:u0 + w], sg[i0][0:64, 0:w])
                                nc.sync.dma_start(qcat_t[2 * mt + 1][64:128, u0:u0 + w], sg[i0][64:128, 0:w])
                            else:
                                # SBUF->SBUF partition-shuffling DMAs into Kc
                                nc.sync.dma_start(Kc[2 * mt][0:64, u0:u0 + w], sg[r0][0:64, 0:w])
                                nc.sync.dma_start(Kc[2 * mt + 1][0:64, u0:u0 + w], sg[r0][64:128, 0:w])
                                nc.sync.dma_start(Kc[2 * mt][64:128, u0:u0 + w], sg[i0][0:64, 0:w])
                                nc.sync.dma_start(Kc[2 * mt + 1][64:128, u0:u0 + w], sg[i0][64:128, 0:w])

                        # V projection (kv path only) -> directly into Vc
                        if not is_q:
                            for vti, (m0, msz) in enumerate(MTI):
                                if not (m0 >= u0 and m0 + msz <= u0 + w):
                                    continue
                                mr = m0 - u0
                                pvr = dps.tile([P, 512], F32, tag="pvr", name="pvr")
                                pvi = dps.tile([P, 512], F32, tag="pvi", name="pvi")
                                for ec in range(ET):
                                    nc.tensor.matmul(pvr[0:msz, :], xr[ec][:, mr:mr + msz],
                                                     Wt["vr"][ec][:],
                                                     start=(ec == 0), stop=(ec == ET - 1))
                                    nc.tensor.matmul(pvi[0:msz, :], xi[ec][:, mr:mr + msz],
                                                     Wt["vi"][ec][:],
                                                     start=(ec == 0), stop=(ec == ET - 1))
                                vco = Vc[vti][0:msz, :].rearrange("p (h c) -> p h c", h=NH)
                                nc.vector.tensor_add(
                                    vco[:, :, 0:64],
                                    pvr[0:msz, :].rearrange("p (h c) -> p h c", h=NH),
                                    vbias["vr"][0:msz, :].rearrange("p (h c) -> p h c", h=NH))
                                nc.vector.tensor_add(
                                    vco[:, :, 64:128],
                                    pvi[0:msz, :].rearrange("p (h c) -> p h c", h=NH),
                                    vbias["vi"][0:msz, :].rearrange("p (h c) -> p h c", h=NH))
                                nc.vector.memset(vco[:, :, 128:129], 1.0)

            # ================= Phase 1 =================
            with tc.tile_pool(name="qn", bufs=1) as qnp, \
                 tc.tile_pool(name="fsl", bufs=1) as fslp, \
                 tc.tile_pool(name="qf", bufs=1) as qfp, \
                 tc.tile_pool(name="stg", bufs=2) as stg:
                # DFT basis slabs (shared by kv and q paths), loaded once,
                # split per j-chunk so only jc0 halves gate the first MMs
                slabs = {nm: [None, None] for nm in ("ce", "se", "co", "so")}

                def load_slabs(jc):
                    j0, jsz = JCH[jc]
                    for nm in ("ce", "se", "co", "so"):
                        t = fslp.tile([P, LH * jsz], R, tag=f"{nm}{jc}", name=f"{nm}{jc}")
                        nc.sync.dma_start(
                            t[:], slab_d[nm].ap()[:, JOFF[jc]:JOFF[jc] + LH * jsz])
                        slabs[nm][jc] = t

                qn_t = [qnp.tile([P, E], R, tag=f"qn{lc}", name=f"qn{lc}")
                        for lc in range(LT)]

                # ---- Phase 1a: kv path (kv tiles streamed per (jc, eb)) ----
                with tc.tile_pool(name="kvs", bufs=1) as kvp, \
                     tc.tile_pool(name="wkv", bufs=1) as wkv, \
                     tc.tile_pool(name="dpsa", bufs=1, space="PSUM") as dpsa:
                    def load_kvt(eb):
                        t = kvp.tile([P, LT * P], R, tag=f"kvs{eb % 3}",
                                     name=f"kvs{eb % 3}")
                        nc.sync.dma_start(t[:], kvt_d.ap()[eb * P:(eb + 1) * P, :])
                        return t
                    # startup-critical DMA order: kvt0, jc0 slabs, kvt1-2,
                    # jc1 slabs, then weights
                    kv_pre = {0: load_kvt(0)}
                    load_slabs(0)
                    kv_pre[1] = load_kvt(1)
                    kv_pre[2] = load_kvt(2)
                    load_slabs(1)

                    def get_xe_kv(eb):
                        t = kv_pre.pop(eb) if eb in kv_pre else load_kvt(eb)
                        return lambda lc: t[:, lc * P:(lc + 1) * P]
                    Wt = {}
                    bias_t = {}
                    for nm in ("kr", "ki", "vr", "vi"):
                        Wt[nm] = [wkv.tile([P, 512], R, tag=f"W{nm}{ec}", name=f"W{nm}{ec}")
                                  for ec in range(ET)]
                        for ec in range(ET):
                            nc.sync.dma_start(Wt[nm][ec][:],
                                                W_d[nm].ap()[ec * P:(ec + 1) * P, :])
                        if nm in ("kr", "ki"):
                            bias_t[nm] = [wkv.tile([P, 1], F32, tag=f"b{nm}{mt}", name=f"b{nm}{mt}")
                                          for mt in range(4)]
                            for mt in range(4):
                                nc.sync.dma_start(bias_t[nm][mt][:],
                                                    W_d["b" + nm].ap()[mt * P:(mt + 1) * P, :])
                    vb_row = wkv.tile([1, 512], F32, tag="vbrow", name="vbrow")
                    vbias = {}
                    for nm in ("vr", "vi"):
                        nc.sync.dma_start(vb_row[:], W_d["b" + nm].ap().rearrange("e one -> one e"))
                        vb = wkv.tile([P, 512], F32, tag=f"vb{nm}", name=f"vb{nm}")
                        nc.gpsimd.partition_broadcast(vb[:], vb_row[:])
                        vbias[nm] = vb

                    dft_proj(False, get_xe_kv,
                             Wt, bias_t, vbias, slabs, dpsa, qfp, stg, None)

                    # LN of q (emitted after kv work: q DMAs queue behind
                    # kv-phase loads; DVE/ACT fill in around kv evictions)
                    with tc.tile_pool(name="ln", bufs=2) as ln, \
                         tc.tile_pool(name="lns", bufs=4) as lns:
                        for lc in range(LT):
                            qt = ln.tile([P, E], F32, tag="qt", name="qt")
                            nc.sync.dma_start(qt[:], q_d.ap()[lc * P:(lc + 1) * P, :])
                            st = lns.tile([P, 12], F32, tag="st", name="st")
                            nc.vector.bn_stats(st[:, 0:6], qt[:, 0:512])
                            nc.vector.bn_stats(st[:, 6:12], qt[:, 512:1024])
                            mv = lns.tile([P, 2], F32, tag="mv", name="mv")
                            nc.vector.bn_aggr(mv[:], st[:])
                            sd = lns.tile([P, 1], F32, tag="sd", name="sd")
                            nc.scalar.activation(sd[:], mv[:, 1:2], AF.Sqrt, bias=eps_t[:])
                            istd = lns.tile([P, 1], F32, tag="istd", name="istd")
                            nc.vector.reciprocal(istd[:], sd[:])
                            nmu = lns.tile([P, 1], F32, tag="nmu", name="nmu")
                            nc.vector.tensor_scalar_mul(nmu[:], mv[:, 0:1], -1.0)
                            nc.vector.tensor_mul(nmu[:], nmu[:], istd[:])
                            nc.scalar.activation(qn_t[lc][:], qt[:], AF.Identity,
                                                 bias=nmu[:], scale=istd[:])

                # ---- Phase 1b: q path ----
                with tc.tile_pool(name="wq", bufs=1) as wq, \
                     tc.tile_pool(name="dpsb", bufs=1, space="PSUM") as dpsb:
                    Wtq = {}
                    bias_q = {}
                    dct = {}
                    for nm in ("qr", "qi"):
                        Wtq[nm] = [wq.tile([P, 512], R, tag=f"W{nm}{ec}", name=f"W{nm}{ec}")
                                   for ec in range(ET)]
                        for ec in range(ET):
                            nc.sync.dma_start(Wtq[nm][ec][:],
                                                W_d[nm].ap()[ec * P:(ec + 1) * P, :])
                        bias_q[nm] = [wq.tile([P, 1], F32, tag=f"b{nm}{mt}", name=f"b{nm}{mt}")
                                      for mt in range(4)]
                        dct[nm] = [wq.tile([P, 1], F32, tag=f"dc{nm}{mt}", name=f"dc{nm}{mt}")
                                   for mt in range(4)]
                        for mt in range(4):
                            nc.sync.dma_start(bias_q[nm][mt][:],
                                                W_d["b" + nm].ap()[mt * P:(mt + 1) * P, :])
                            nc.sync.dma_start(dct[nm][mt][:],
                                                dc_d[nm].ap()[mt * P:(mt + 1) * P, :])
                    dft_proj(True,
                             lambda eb: (lambda lc: qn_t[lc][:, eb * P:(eb + 1) * P]),
                             Wtq, bias_q, None, slabs, dpsb, qfp, stg, dct)

            # ================= Phase 2: attention =================
            attn_ctx = [tc.tile_pool(name="qk", bufs=1),
                        tc.tile_pool(name="expp", bufs=3),
                        tc.tile_pool(name="sps", bufs=4, space="PSUM"),
                        tc.tile_pool(name="avps", bufs=3, space="PSUM"),
                        tc.tile_pool(name="nrm", bufs=4)]
            qk, expp, sps, avps, nrm = [c.__enter__() for c in attn_ctx]
            Qc = []
            for h in range(NH):
                qt = qk.tile([P, FP], R, tag=f"Qc{h}", name=f"Qc{h}")
                nc.sync.dma_start(qt[:], qcat_t[h][:, :])
                Qc.append(qt)

            def do_av(h, expts):
                for ti, (l0, lsz) in enumerate(MTI):
                    ps = avps.tile([P, 129], F32, tag="av", name="av")
                    n = len(MTI)
                    for mi, (m0, msz) in enumerate(MTI):
                        nc.tensor.matmul(ps[0:lsz, :], expts[mi][0:msz, l0:l0 + lsz],
                                         Vc[mi][0:msz, h * 129:(h + 1) * 129],
                                         start=(mi == 0), stop=(mi == n - 1))
                    rcp = nrm.tile([P, 1], F32, tag="rcp", name="rcp")
                    nc.vector.reciprocal(rcp[0:lsz, :], ps[0:lsz, 128:129])
                    nc.vector.tensor_scalar_mul(our[ti][0:lsz, h * 64:(h + 1) * 64],
                                                ps[0:lsz, 0:64], rcp[0:lsz, :])
                    nc.vector.tensor_scalar_mul(oui[ti][0:lsz, h * 64:(h + 1) * 64],
                                                ps[0:lsz, 64:128], rcp[0:lsz, :])

            # software pipeline: AV for head h-1 overlaps scores/exp for h
            prev = None
            for h in range(NH):
                expts = []
                for ti, (m0, msz) in enumerate(MTI):
                    et_ = expp.tile([P, FP], BF16, tag=f"exp{ti}", name=f"exp{ti}")
                    for (f0, fsz) in FCH:
                        ps = sps.tile([P, 384], F32, tag="sc", name="sc")
                        nc.tensor.matmul(ps[0:msz, 0:fsz], Kc[h][:, m0:m0 + msz],
                                         Qc[h][:, f0:f0 + fsz], start=True, stop=True)
                        nc.scalar.activation(et_[0:msz, f0:f0 + fsz], ps[0:msz, 0:fsz],
                                             AF.Exp, scale=float(D ** -0.5))
                    expts.append(et_)
                if prev is not None:
                    do_av(h - 1, prev)
                prev = expts
            do_av(NH - 1, prev)
            for c in reversed(attn_ctx):
                c.__exit__(None, None, None)

            # ================= Phase 3: radix-2 iDFT + Wo =================
            # OTT columns hold [even times tau 0..1023 | odd times]; the
            # final output DMA de-interleaves via a strided DRAM view.
            with tc.tile_pool(name="gsl", bufs=3) as gsl, \
                 tc.tile_pool(name="uvp", bufs=1) as uvp, \
                 tc.tile_pool(name="ott", bufs=1) as ottp, \
                 tc.tile_pool(name="wop", bufs=1) as wop, \
                 tc.tile_pool(name="ost", bufs=3) as ost:
                OTT = [ottp.tile([P, L], R, tag=f"OTT{i}", name=f"OTT{i}") for i in range(4)]
                WoT_t = [wop.tile([P, E], R, tag=f"wo{i}", name=f"wo{i}") for i in range(4)]
                for ec in range(4):
                    nc.sync.dma_start(WoT_t[ec][:], WoT_d.ap()[ec * P:(ec + 1) * P, :])
                # pre-halve DC (u=0) and Nyquist (u=513 -> k=1024) rows;
                # basis matrices carry the x2 weighting for all rows
                for t_ in (our[0], oui[0], our[5], oui[5]):
                    nc.vector.tensor_scalar_mul(t_[0:1, :], t_[0:1, :], 0.5)
                Ur, Ui, Vr, Vi = [], [], [], []
                for jt in range(4):
                    u_r = uvp.tile([P, 512], R, tag=f"Ur{jt}", name=f"Ur{jt}")
                    u_i = uvp.tile([P, 512], R, tag=f"Ui{jt}", name=f"Ui{jt}")
                    v_r = uvp.tile([P, 512], R, tag=f"Vr{jt}", name=f"Vr{jt}")
                    v_i = uvp.tile([P, 512], R, tag=f"Vi{jt}", name=f"Vi{jt}")
                    nc.vector.tensor_add(u_r[:], our[jt][:], our[5 + jt][:])
                    nc.vector.tensor_sub(v_r[:], our[jt][:], our[5 + jt][:])
                    nc.vector.tensor_sub(u_i[:], oui[jt][:], oui[5 + jt][:])
                    nc.vector.tensor_add(v_i[:], oui[jt][:], oui[5 + jt][:])
                    Ur.append(u_r)
                    Ui.append(u_i)
                    Vr.append(v_r)
                    Vi.append(v_i)

                out_v = out_d.ap().rearrange("(t two) e -> two t e", two=2)
                idps_ctx = tc.tile_pool(name="idps", bufs=1, space="PSUM")
                idps = idps_ctx.__enter__()
                wops_ctx = tc.tile_pool(name="wops", bufs=2, space="PSUM")
                wops = wops_ctx.__enter__()

                def wo_block(tb):
                    pso = [wops.tile([P, 512], F32, tag=f"po{eo}", name=f"po{eo}")
                           for eo in range(2)]
                    for eo in range(2):
                        for ec in range(4):
                            nc.tensor.matmul(pso[eo][:],
                                             OTT[ec][:, tb * P:(tb + 1) * P],
                                             WoT_t[ec][:, eo * 512:(eo + 1) * 512],
                                             start=(ec == 0), stop=(ec == 3))
                    ot_ = ost.tile([P, E], F32, tag="ot", name="ot")
                    nc.vector.tensor_copy(ot_[:, 0:512], pso[0][:])
                    nc.scalar.activation(ot_[:, 512:1024], pso[1][:], AF.Copy)
                    nc.sync.dma_start(
                        out_v[tb // 8, (tb % 8) * P:(tb % 8 + 1) * P, :], ot_[:])

                # 4-bank iDFT passes (oh x e4-pair) so Wo for the even half
                # overlaps the odd-half iDFT on the PE
                for oh in range(2):
                    nm_c, nm_s = ("iec", "ies") if oh == 0 else ("ioc", "ios")
                    mct = []
                    mst = []
                    for jt in range(5):
                        msz = P if jt < 4 else 1
                        r0 = jt * P
                        mc = gsl.tile([P, 1024], R, tag=f"gc{jt}", name=f"gc{jt}", bufs=2)
                        ms = gsl.tile([P, 1024], R, tag=f"gs{jt}", name=f"gs{jt}", bufs=2)
                        nc.sync.dma_start(mc[0:msz, :], ig_d[nm_c].ap()[r0:r0 + msz, :])
                        nc.sync.dma_start(ms[0:msz, :], ig_d[nm_s].ap()[r0:r0 + msz, :])
                        mct.append(mc)
                        mst.append(ms)
                    for ep in range(2):
                        pst = [[idps.tile([P, 512], F32, tag=f"ph{i}_{t2}", name=f"ph{i}_{t2}")
                                for t2 in range(2)] for i in range(2)]
                        for jt in range(5):
                            msz = P if jt < 4 else 1
                            mc = mct[jt]
                            ms = mst[jt]
                            if jt < 4:
                                sr = Ur[jt] if oh == 0 else Vr[jt]
                                si = Ui[jt] if oh == 0 else Vi[jt]
                            else:
                                sr, si = our[4], oui[4]
                            for i in range(2):
                                e4 = 2 * ep + i
                                for t2 in range(2):
                                    nc.tensor.matmul(pst[i][t2][:],
                                                     sr[0:msz, e4 * P:(e4 + 1) * P],
                                                     mc[0:msz, t2 * 512:(t2 + 1) * 512],
                                                     start=(jt == 0), stop=False)
                                    nc.tensor.matmul(pst[i][t2][:],
                                                     si[0:msz, e4 * P:(e4 + 1) * P],
                                                     ms[0:msz, t2 * 512:(t2 + 1) * 512],
                                                     start=False, stop=(jt == 4))
                        for i in range(2):
                            e4 = 2 * ep + i
                            for t2 in range(2):
                                dst = OTT[e4][:, oh * 1024 + t2 * 512:oh * 1024 + (t2 + 1) * 512]
                                if i == 0:
                                    nc.vector.tensor_copy(dst, pst[i][t2][:])
                                else:
                                    nc.scalar.activation(dst, pst[i][t2][:], AF.Copy)
                    if oh == 0:
                        for tb in range(8):
                            wo_block(tb)
                for tb in range(8, LT):
                    wo_block(tb)
                wops_ctx.__exit__(None, None, None)
                idps_ctx.__exit__(None, None, None)

    nc.finalize()
    return nc


def kernel(**inputs):
    from concourse.bass_utils import run_bass_kernel_spmd

    rdt = ml_dtypes.bfloat16 if MM_BF16 else np.float32
    if "nc" not in _CACHE:
        _CACHE["nc"] = _build()
        Ce, Se, Co, So, Ec, Es, Oc, Os = _dft_consts()
        _CACHE["consts"] = {
            "ce": _pretile_half(Ce).astype(rdt),
            "se": _pretile_half(Se).astype(rdt),
            "co": _pretile_half(Co).astype(rdt),
            "so": _pretile_half(So).astype(rdt),
            "iec": Ec.astype(rdt), "ies": Es.astype(rdt),
            "ioc": Oc.astype(rdt), "ios": Os.astype(rdt),
        }
    nc = _CACHE["nc"]
    C = _CACHE["consts"]

    perm = np.r_[0:L:2, 1:L:2]
    q = np.ascontiguousarray(inputs["query"], dtype=np.float32)[:, perm, :]
    kv = np.asarray(inputs["key_value"], dtype=np.float32)[:, perm, :]
    gamma = np.asarray(inputs["gamma"], np.float32)
    beta = np.asarray(inputs["beta"], np.float32)
    in_maps = []
    for core in range(8):
        b = core // 2
        hg = core % 2
        cs = slice(hg * 512, (hg + 1) * 512)
        kvt = np.ascontiguousarray(
            kv[b].reshape(LT, P, ET, P).transpose(2, 1, 0, 3).reshape(ET * P, LT * P)
        ).astype(rdt)
        m = {
            "q": np.ascontiguousarray(q[b]),
            "kvt": kvt,
            "ce": C["ce"], "se": C["se"], "co": C["co"], "so": C["so"],
            "iec": C["iec"], "ies": C["ies"], "ioc": C["ioc"], "ios": C["ios"],
            "WoT": np.ascontiguousarray(inputs["Wo"][:, cs].T.astype(rdt)),
        }
        for nm in ("qr", "qi", "kr", "ki", "vr", "vi"):
            Wcs = np.asarray(inputs["W" + nm], np.float32)[cs, :]
            if nm in ("qr", "qi"):
                m[f"dc{nm}"] = np.ascontiguousarray(
                    (Wcs @ beta) * math.sqrt(L), np.float32).reshape(512, 1)
                Wcs = Wcs * gamma[None, :]
            m[f"W{nm}"] = np.ascontiguousarray(Wcs.T.astype(rdt))
            m[f"b{nm}"] = np.ascontiguousarray(inputs["b" + nm][cs], np.float32).reshape(512, 1)
        in_maps.append(m)

    res = run_bass_kernel_spmd(nc, in_maps, core_ids=list(range(8)))
    _CACHE["last"] = res
    out = np.empty((B, L, E), np.float32)
    for b in range(B):
        out[b] = res.results[2 * b]["out"] + res.results[2 * b + 1]["out"]
    return out


# revision 42
# speedup vs baseline: 1.0630x; 1.0068x over previous
"""Trainium2 Bass kernel for nn_FreqCrossAttention.

Sharding: 8 cores = 4 batches x 2 head-groups (8 heads each).
Each core computes a partial output [2048, 1024] (its head-group's
contribution through W_o row-parallel); host sums the pair per batch.

v3: radix-2 DFT. Host permutes the L axis into (even, odd) halves; the
rfft becomes two 1024-point half-DFTs (twiddles folded into the odd
basis matrices) plus a 4-op combine:
    A (freqs k=0..512)        = E + P
    B (freqs k=1024..513)     : Br = Er - Pr, Bi = Pi - Ei
Frequencies are stored in "split order" [A | B]; the inverse
permutation is folded into the host-built iDFT matrices. Gamma is
folded into Wqr/Wqi host-side; beta's DC term is added on-device to
storage column u=0 post-projection. DFT matmul FLOPs halve.
"""
import math
import numpy as np
import ml_dtypes

MM_BF16 = True

B, L, E, H = 4, 2048, 1024, 16
D = E // H            # 64
Lf = L // 2 + 1       # 1025
FP = 1026             # padded frequency dim (split-order + 1 pad col)
NH = 8                # heads per core
P = 128
LH = 8                # L-tiles per half (even/odd)
JCH = [(0, 256), (256, 257)]          # j-chunks of the 513 half-freqs
JOFF = [0, LH * 256]                  # free offsets in pretiled slabs
# storage chunks: (u0, width, j-chunk index, block)
SCH = [(0, 256, 0, 'A'), (256, 257, 1, 'A'),
       (513, 256, 0, 'B'), (769, 256, 1, 'B')]
# m-tiles in storage order (1025 real bins; singleton at u=512);
# also used as l-tiles for AV and j-tiles for the radix-2 iDFT
MTI = [(0, 128), (128, 128), (256, 128), (384, 128), (512, 1),
       (513, 128), (641, 128), (769, 128), (897, 128)]
FCH = [(0, 384), (384, 384), (768, 257)]   # scores moving-dim chunks
ET = 8                # e-chunks of E
LT = 16               # L tiles
EPS = 1e-5

_CACHE = {}


def _pretile_half(M):
    # [1024, 513] -> [P, LH*513] chunk-major (j-chunks of JCH)
    r = M.reshape(LH, P, 513)
    blocks = []
    for (j0, jsz) in JCH:
        blocks.append(np.ascontiguousarray(
            r[:, :, j0:j0 + jsz].transpose(1, 0, 2).reshape(P, LH * jsz)))
    return np.concatenate(blocks, axis=1)              # [P, LH*513]


def _dft_consts():
    s = 1.0 / math.sqrt(L)
    m = np.arange(1024)[:, None].astype(np.float64)
    j = np.arange(513)[None, :].astype(np.float64)
    ang_e = 2.0 * np.pi * (2.0 * m) * j / L
    ang_o = 2.0 * np.pi * (2.0 * m + 1.0) * j / L
    Ce = (np.cos(ang_e) * s).astype(np.float32)
    Se = (-np.sin(ang_e) * s).astype(np.float32)
    Co = (np.cos(ang_o) * s).astype(np.float32)
    So = (-np.sin(ang_o) * s).astype(np.float32)
    # radix-2 iDFT half-matrices (cw doubling pre-folded: rows x2; the
    # DC/Nyquist rows of our/oui get pre-halved on device instead)
    tau = np.arange(1024)[None, :].astype(np.float64)
    jj = np.arange(513)[:, None].astype(np.float64)
    Ec = (2.0 * s * np.cos(2.0 * np.pi * jj * tau / 1024.0)).astype(np.float32)
    Es = (-2.0 * s * np.sin(2.0 * np.pi * jj * tau / 1024.0)).astype(np.float32)
    Oc = (2.0 * s * np.cos(np.pi * jj * (2.0 * tau + 1.0) / 1024.0)).astype(np.float32)
    Os = (-2.0 * s * np.sin(np.pi * jj * (2.0 * tau + 1.0) / 1024.0)).astype(np.float32)
    return Ce, Se, Co, So, Ec, Es, Oc, Os


def _build():
    import concourse.bass as bass
    import concourse.bacc as bacc
    import concourse.mybir as mybir
    import concourse.tile as tile

    R = mybir.dt.bfloat16 if MM_BF16 else mybir.dt.float32r
    F32 = mybir.dt.float32
    BF16 = mybir.dt.bfloat16
    AF = mybir.ActivationFunctionType

    nc = bacc.Bacc("TRN2", debug=False, num_devices=8)

    q_d = nc.dram_tensor("q", [L, E], R, kind="ExternalInput")
    kvt_d = nc.dram_tensor("kvt", [ET * P, LT * P], R, kind="ExternalInput")
    slab_d = {}
    for nm in ("ce", "se", "co", "so"):
        slab_d[nm] = nc.dram_tensor(nm, [P, LH * 513], R, kind="ExternalInput")
    ig_d = {}
    for nm in ("iec", "ies", "ioc", "ios"):
        ig_d[nm] = nc.dram_tensor(nm, [513, 1024], R, kind="ExternalInput")
    W_d = {}
    for nm in ("qr", "qi", "kr", "ki", "vr", "vi"):
        W_d[nm] = nc.dram_tensor(f"W{nm}", [E, 512], R, kind="ExternalInput")
        W_d["b" + nm] = nc.dram_tensor(f"b{nm}", [512, 1], F32, kind="ExternalInput")
    dc_d = {nm: nc.dram_tensor(f"dc{nm}", [512, 1], F32, kind="ExternalInput")
            for nm in ("qr", "qi")}
    WoT_d = nc.dram_tensor("WoT", [512, E], R, kind="ExternalInput")
    out_d = nc.dram_tensor("out", [L, E], F32, kind="ExternalOutput")

    with tile.TileContext(nc) as tc:
        with tc.tile_pool(name="dram", bufs=1, space="DRAM") as dram, \
             tc.tile_pool(name="persist", bufs=1) as persist, \
             tc.tile_pool(name="kcl", bufs=1) as kcl, \
             tc.tile_pool(name="vcl", bufs=1) as vcl, \
             tc.tile_pool(name="oacc", bufs=1) as oacc:
            qcat_t = [dram.tile([P, FP], R, tag=f"qcat{h}", name=f"qcat{h}")
                      for h in range(NH)]

            eps_t = persist.tile([P, 1], F32)
            nc.vector.memset(eps_t[:], EPS)

            # K and V stay SBUF-resident from projection through attention
            Kc = [kcl.tile([P, FP], R, tag=f"Kc{h}", name=f"Kc{h}")
                  for h in range(NH)]
            Vc = [vcl.tile([P, NH * 129], BF16, tag=f"Vc{ti}", name=f"Vc{ti}")
                  for ti in range(len(MTI))]

            our = []
            oui = []
            for ti in range(len(MTI)):
                our.append(oacc.tile([P, 512], R, tag=f"our{ti}", name=f"our{ti}"))
                oui.append(oacc.tile([P, 512], R, tag=f"oui{ti}", name=f"oui{ti}"))

            # ---------- radix-2 DFT + projection worker ----------
            def dft_proj(is_q, get_xe, Wt, bias_t, vbias, slabs, dps, qfp, stg, dct):
                pnames = ("qr", "qi") if is_q else ("kr", "ki")
                for jc, (j0, jsz) in enumerate(JCH):
                    bw = 256   # B-block width in this chunk
                    xa_r, xa_i, xb_r, xb_i = [], [], [], []
                    for eb in range(ET):
                        xe = get_xe(eb)
                        pEr = dps.tile([P, jsz], F32, tag="pEr", name="pEr")
                        pEi = dps.tile([P, jsz], F32, tag="pEi", name="pEi")
                        pPr = dps.tile([P, jsz], F32, tag="pPr", name="pPr")
                        pPi = dps.tile([P, jsz], F32, tag="pPi", name="pPi")
                        for lh in range(LH):
                            sl = slice(lh * jsz, (lh + 1) * jsz)
                            st_, sp_ = (lh == 0), (lh == LH - 1)
                            nc.tensor.matmul(pEr[:], xe(lh), slabs["ce"][jc][:, sl],
                                             start=st_, stop=sp_)
                            nc.tensor.matmul(pEi[:], xe(lh), slabs["se"][jc][:, sl],
                                             start=st_, stop=sp_)
                            nc.tensor.matmul(pPr[:], xe(LH + lh), slabs["co"][jc][:, sl],
                                             start=st_, stop=sp_)
                            nc.tensor.matmul(pPi[:], xe(LH + lh), slabs["so"][jc][:, sl],
                                             start=st_, stop=sp_)
                        # E -> SBUF via ACT, then 4 DVE combines (SBUF, PSUM)
                        erS = stg.tile([P, 257], F32, tag="erS", name="erS", bufs=2)
                        eiS = stg.tile([P, 257], F32, tag="eiS", name="eiS", bufs=2)
                        nc.scalar.activation(erS[:, 0:jsz], pEr[:], AF.Copy)
                        nc.scalar.activation(eiS[:, 0:jsz], pEi[:], AF.Copy)
                        ar = qfp.tile([P, jsz], R, tag=f"ar{eb}", name=f"ar{eb}")
                        ai = qfp.tile([P, jsz], R, tag=f"ai{eb}", name=f"ai{eb}")
                        br = qfp.tile([P, bw], R, tag=f"br{eb}", name=f"br{eb}")
                        bi = qfp.tile([P, bw], R, tag=f"bi{eb}", name=f"bi{eb}")
                        nc.vector.tensor_add(ar[:], erS[:, 0:jsz], pPr[:])
                        nc.vector.tensor_add(ai[:], eiS[:, 0:jsz], pPi[:])
                        nc.vector.tensor_sub(br[:], erS[:, 0:bw], pPr[:, 0:bw])
                        nc.vector.tensor_sub(bi[:], pPi[:, 0:bw], eiS[:, 0:bw])
                        xa_r.append(ar)
                        xa_i.append(ai)
                        xb_r.append(br)
                        xb_i.append(bi)

                    for (u0, w, jcc, kind) in SCH:
                        if jcc != jc:
                            continue
                        xr = xa_r if kind == 'A' else xb_r
                        xi = xa_i if kind == 'A' else xb_i
                        # Q/K projections for this storage chunk
                        for mt in range(4):
                            pps = {}
                            for nm in pnames:
                                pps[nm] = dps.tile([P, w], F32, tag=f"pp{nm}", name=f"pp{nm}")
                            for ec in range(ET):
                                src = {pnames[0]: xr[ec], pnames[1]: xi[ec]}
                                for nm in pnames:
                                    nc.tensor.matmul(pps[nm][:],
                                                     Wt[nm][ec][:, mt * P:(mt + 1) * P],
                                                     src[nm][:],
                                                     start=(ec == 0), stop=(ec == ET - 1))
                            sg = {}
                            for nm in pnames:
                                s = stg.tile([P, 257], R, tag=f"sg{nm}", name=f"sg{nm}", bufs=2)
                                nc.scalar.activation(s[:, 0:w], pps[nm][:], AF.Identity,
                                                     bias=bias_t[nm][mt][:])
                                if is_q and u0 == 0:
                                    # beta's DC contribution (storage col u=0)
                                    nc.vector.scalar_tensor_tensor(
                                        s[:, 0:1], dct[nm][mt][:], 1.0,
                                        s[:, 0:1],
                                        op0=mybir.AluOpType.mult,
                                        op1=mybir.AluOpType.add)
                                sg[nm] = s
                            r0, i0 = pnames
                            if is_q:
                                nc.sync.dma_start(qcat_t[2 * mt][0:64, u0:u0 + w], sg[r0][0:64, 0:w])
                                nc.sync.dma_start(qcat_t[2 * mt + 1][0:64, u0:u0 + w], sg[r0][64:128, 0:w])
                                nc.sync.dma_start(qcat_t[2 * mt][64:128, u0:u0 + w], sg[i0][0:64, 0:w])
                                nc.sync.dma_start(qcat_t[2 * mt + 1][64:128, u0:u0 + w], sg[i0][64:128, 0:w])
                            else:
                                # SBUF->SBUF partition-shuffling DMAs into Kc
                                nc.sync.dma_start(Kc[2 * mt][0:64, u0:u0 + w], sg[r0][0:64, 0:w])
                                nc.sync.dma_start(Kc[2 * mt + 1][0:64, u0:u0 + w], sg[r0][64:128, 0:w])
                                nc.sync.dma_start(Kc[2 * mt][64:128, u0:u0 + w], sg[i0][0:64, 0:w])
                                nc.sync.dma_start(Kc[2 * mt + 1][64:128, u0:u0 + w], sg[i0][64:128, 0:w])

                        # V projection (kv path only) -> directly into Vc
                        if not is_q:
                            for vti, (m0, msz) in enumerate(MTI):
                                if not (m0 >= u0 and m0 + msz <= u0 + w):
                                    continue
                                mr = m0 - u0
                                pvr = dps.tile([P, 512], F32, tag="pvr", name="pvr")
                                pvi = dps.tile([P, 512], F32, tag="pvi", name="pvi")
                                for ec in range(ET):
                                    nc.tensor.matmul(pvr[0:msz, :], xr[ec][:, mr:mr + msz],
                                                     Wt["vr"][ec][:],
                                                     start=(ec == 0), stop=(ec == ET - 1))
                                    nc.tensor.matmul(pvi[0:msz, :], xi[ec][:, mr:mr + msz],
                                                     Wt["vi"][ec][:],
                                                     start=(ec == 0), stop=(ec == ET - 1))
                                vco = Vc[vti][0:msz, :].rearrange("p (h c) -> p h c", h=NH)
                                nc.vector.tensor_add(
                                    vco[:, :, 0:64],
                                    pvr[0:msz, :].rearrange("p (h c) -> p h c", h=NH),
                                    vbias["vr"][0:msz, :].rearrange("p (h c) -> p h c", h=NH))
                                nc.vector.tensor_add(
                                    vco[:, :, 64:128],
                                    pvi[0:msz, :].rearrange("p (h c) -> p h c", h=NH),
                                    vbias["vi"][0:msz, :].rearrange("p (h c) -> p h c", h=NH))
                                nc.vector.memset(vco[:, :, 128:129], 1.0)

            # ================= Phase 1 =================
            with tc.tile_pool(name="qn", bufs=1) as qnp, \
                 tc.tile_pool(name="fsl", bufs=1) as fslp, \
                 tc.tile_pool(name="qf", bufs=1) as qfp, \
                 tc.tile_pool(name="stg", bufs=2) as stg:
                # DFT basis slabs (shared by kv and q paths), loaded once,
                # split per j-chunk so only jc0 halves gate the first MMs
                slabs = {nm: [None, None] for nm in ("ce", "se", "co", "so")}

                def load_slabs(jc):
                    j0, jsz = JCH[jc]
                    for nm in ("ce", "se", "co", "so"):
                        t = fslp.tile([P, LH * jsz], R, tag=f"{nm}{jc}", name=f"{nm}{jc}")
                        nc.sync.dma_start(
                            t[:], slab_d[nm].ap()[:, JOFF[jc]:JOFF[jc] + LH * jsz])
                        slabs[nm][jc] = t

                qn_t = [qnp.tile([P, E], R, tag=f"qn{lc}", name=f"qn{lc}")
                        for lc in range(LT)]

                # ---- Phase 1a: kv path (kv tiles streamed per (jc, eb)) ----
                with tc.tile_pool(name="kvs", bufs=1) as kvp, \
                     tc.tile_pool(name="wkv", bufs=1) as wkv, \
                     tc.tile_pool(name="dpsa", bufs=1, space="PSUM") as dpsa:
                    def load_kvt(eb):
                        t = kvp.tile([P, LT * P], R, tag=f"kvs{eb % 3}",
                                     name=f"kvs{eb % 3}")
                        nc.sync.dma_start(t[:], kvt_d.ap()[eb * P:(eb + 1) * P, :])
                        return t
                    # startup-critical DMA order: kvt0, jc0 slabs, kvt1-2,
                    # jc1 slabs, then weights
                    kv_pre = {0: load_kvt(0)}
                    load_slabs(0)
                    kv_pre[1] = load_kvt(1)
                    kv_pre[2] = load_kvt(2)
                    load_slabs(1)

                    def get_xe_kv(eb):
                        t = kv_pre.pop(eb) if eb in kv_pre else load_kvt(eb)
                        return lambda lc: t[:, lc * P:(lc + 1) * P]
                    Wt = {}
                    bias_t = {}
                    for nm in ("kr", "ki", "vr", "vi"):
                        Wt[nm] = [wkv.tile([P, 512], R, tag=f"W{nm}{ec}", name=f"W{nm}{ec}")
                                  for ec in range(ET)]
                        for ec in range(ET):
                            nc.sync.dma_start(Wt[nm][ec][:],
                                                W_d[nm].ap()[ec * P:(ec + 1) * P, :])
                        if nm in ("kr", "ki"):
                            bias_t[nm] = [wkv.tile([P, 1], F32, tag=f"b{nm}{mt}", name=f"b{nm}{mt}")
                                          for mt in range(4)]
                            for mt in range(4):
                                nc.sync.dma_start(bias_t[nm][mt][:],
                                                    W_d["b" + nm].ap()[mt * P:(mt + 1) * P, :])
                    vb_row = wkv.tile([1, 512], F32, tag="vbrow", name="vbrow")
                    vbias = {}
                    for nm in ("vr", "vi"):
                        nc.sync.dma_start(vb_row[:], W_d["b" + nm].ap().rearrange("e one -> one e"))
                        vb = wkv.tile([P, 512], F32, tag=f"vb{nm}", name=f"vb{nm}")
                        nc.gpsimd.partition_broadcast(vb[:], vb_row[:])
                        vbias[nm] = vb

                    dft_proj(False, get_xe_kv,
                             Wt, bias_t, vbias, slabs, dpsa, qfp, stg, None)

                    # LN of q (emitted after kv work: q DMAs queue behind
                    # kv-phase loads; DVE/ACT fill in around kv evictions)
                    with tc.tile_pool(name="ln", bufs=2) as ln, \
                         tc.tile_pool(name="lns", bufs=4) as lns:
                        for lc in range(LT):
                            qt = ln.tile([P, E], R, tag="qt", name="qt")
                            nc.sync.dma_start(qt[:], q_d.ap()[lc * P:(lc + 1) * P, :])
                            st = lns.tile([P, 12], F32, tag="st", name="st")
                            nc.vector.bn_stats(st[:, 0:6], qt[:, 0:512])
                            nc.vector.bn_stats(st[:, 6:12], qt[:, 512:1024])
                            mv = lns.tile([P, 2], F32, tag="mv", name="mv")
                            nc.vector.bn_aggr(mv[:], st[:])
                            sd = lns.tile([P, 1], F32, tag="sd", name="sd")
                            nc.scalar.activation(sd[:], mv[:, 1:2], AF.Sqrt, bias=eps_t[:])
                            istd = lns.tile([P, 1], F32, tag="istd", name="istd")
                            nc.vector.reciprocal(istd[:], sd[:])
                            nmu = lns.tile([P, 1], F32, tag="nmu", name="nmu")
                            nc.vector.tensor_scalar_mul(nmu[:], mv[:, 0:1], -1.0)
                            nc.vector.tensor_mul(nmu[:], nmu[:], istd[:])
                            nc.scalar.activation(qn_t[lc][:], qt[:], AF.Identity,
                                                 bias=nmu[:], scale=istd[:])

                # ---- Phase 1b: q path ----
                with tc.tile_pool(name="wq", bufs=1) as wq, \
                     tc.tile_pool(name="dpsb", bufs=1, space="PSUM") as dpsb:
                    Wtq = {}
                    bias_q = {}
                    dct = {}
                    for nm in ("qr", "qi"):
                        Wtq[nm] = [wq.tile([P, 512], R, tag=f"W{nm}{ec}", name=f"W{nm}{ec}")
                                   for ec in range(ET)]
                        for ec in range(ET):
                            nc.sync.dma_start(Wtq[nm][ec][:],
                                                W_d[nm].ap()[ec * P:(ec + 1) * P, :])
                        bias_q[nm] = [wq.tile([P, 1], F32, tag=f"b{nm}{mt}", name=f"b{nm}{mt}")
                                      for mt in range(4)]
                        dct[nm] = [wq.tile([P, 1], F32, tag=f"dc{nm}{mt}", name=f"dc{nm}{mt}")
                                   for mt in range(4)]
                        for mt in range(4):
                            nc.sync.dma_start(bias_q[nm][mt][:],
                                                W_d["b" + nm].ap()[mt * P:(mt + 1) * P, :])
                            nc.sync.dma_start(dct[nm][mt][:],
                                                dc_d[nm].ap()[mt * P:(mt + 1) * P, :])
                    dft_proj(True,
                             lambda eb: (lambda lc: qn_t[lc][:, eb * P:(eb + 1) * P]),
                             Wtq, bias_q, None, slabs, dpsb, qfp, stg, dct)

            # ================= Phase 2: attention =================
            attn_ctx = [tc.tile_pool(name="qk", bufs=1),
                        tc.tile_pool(name="expp", bufs=3),
                        tc.tile_pool(name="sps", bufs=4, space="PSUM"),
                        tc.tile_pool(name="avps", bufs=4, space="PSUM"),
                        tc.tile_pool(name="nrm", bufs=4)]
            qk, expp, sps, avps, nrm = [c.__enter__() for c in attn_ctx]
            Qc = []
            for h in range(NH):
                qt = qk.tile([P, FP], R, tag=f"Qc{h}", name=f"Qc{h}")
                nc.sync.dma_start(qt[:], qcat_t[h][:, :])
                Qc.append(qt)

            def do_av(h, expts):
                for ti, (l0, lsz) in enumerate(MTI):
                    ps = avps.tile([P, 129], F32, tag="av", name="av")
                    n = len(MTI)
                    for mi, (m0, msz) in enumerate(MTI):
                        nc.tensor.matmul(ps[0:lsz, :], expts[mi][0:msz, l0:l0 + lsz],
                                         Vc[mi][0:msz, h * 129:(h + 1) * 129],
                                         start=(mi == 0), stop=(mi == n - 1))
                    rcp = nrm.tile([P, 1], F32, tag="rcp", name="rcp")
                    nc.vector.reciprocal(rcp[0:lsz, :], ps[0:lsz, 128:129])
                    nc.vector.tensor_scalar_mul(our[ti][0:lsz, h * 64:(h + 1) * 64],
                                                ps[0:lsz, 0:64], rcp[0:lsz, :])
                    nc.vector.tensor_scalar_mul(oui[ti][0:lsz, h * 64:(h + 1) * 64],
                                                ps[0:lsz, 64:128], rcp[0:lsz, :])

            # software pipeline: AV for head h-1 overlaps scores/exp for h
            prev = None
            for h in range(NH):
                expts = []
                for ti, (m0, msz) in enumerate(MTI):
                    et_ = expp.tile([P, FP], BF16, tag=f"exp{ti}", name=f"exp{ti}")
                    for (f0, fsz) in FCH:
                        ps = sps.tile([P, 384], F32, tag="sc", name="sc")
                        nc.tensor.matmul(ps[0:msz, 0:fsz], Kc[h][:, m0:m0 + msz],
                                         Qc[h][:, f0:f0 + fsz], start=True, stop=True)
                        nc.scalar.activation(et_[0:msz, f0:f0 + fsz], ps[0:msz, 0:fsz],
                                             AF.Exp, scale=float(D ** -0.5))
                    expts.append(et_)
                if prev is not None:
                    do_av(h - 1, prev)
                prev = expts
            do_av(NH - 1, prev)
            for c in reversed(attn_ctx):
                c.__exit__(None, None, None)

            # ================= Phase 3: radix-2 iDFT + Wo =================
            # OTT columns hold [even times tau 0..1023 | odd times]; the
            # final output DMA de-interleaves via a strided DRAM view.
            with tc.tile_pool(name="gsl", bufs=3) as gsl, \
                 tc.tile_pool(name="uvp", bufs=1) as uvp, \
                 tc.tile_pool(name="ott", bufs=1) as ottp, \
                 tc.tile_pool(name="wop", bufs=1) as wop, \
                 tc.tile_pool(name="ost", bufs=3) as ost:
                OTT = [ottp.tile([P, L], R, tag=f"OTT{i}", name=f"OTT{i}") for i in range(4)]
                WoT_t = [wop.tile([P, E], R, tag=f"wo{i}", name=f"wo{i}") for i in range(4)]
                for ec in range(4):
                    nc.sync.dma_start(WoT_t[ec][:], WoT_d.ap()[ec * P:(ec + 1) * P, :])
                # pre-halve DC (u=0) and Nyquist (u=513 -> k=1024) rows;
                # basis matrices carry the x2 weighting for all rows
                for t_ in (our[0], oui[0], our[5], oui[5]):
                    nc.vector.tensor_scalar_mul(t_[0:1, :], t_[0:1, :], 0.5)
                Ur, Ui, Vr, Vi = [], [], [], []
                for jt in range(4):
                    u_r = uvp.tile([P, 512], R, tag=f"Ur{jt}", name=f"Ur{jt}")
                    u_i = uvp.tile([P, 512], R, tag=f"Ui{jt}", name=f"Ui{jt}")
                    v_r = uvp.tile([P, 512], R, tag=f"Vr{jt}", name=f"Vr{jt}")
                    v_i = uvp.tile([P, 512], R, tag=f"Vi{jt}", name=f"Vi{jt}")
                    nc.vector.tensor_add(u_r[:], our[jt][:], our[5 + jt][:])
                    nc.vector.tensor_sub(v_r[:], our[jt][:], our[5 + jt][:])
                    nc.vector.tensor_sub(u_i[:], oui[jt][:], oui[5 + jt][:])
                    nc.vector.tensor_add(v_i[:], oui[jt][:], oui[5 + jt][:])
                    Ur.append(u_r)
                    Ui.append(u_i)
                    Vr.append(v_r)
                    Vi.append(v_i)

                out_v = out_d.ap().rearrange("(t two) e -> two t e", two=2)
                idps_ctx = tc.tile_pool(name="idps", bufs=1, space="PSUM")
                idps = idps_ctx.__enter__()
                wops_ctx = tc.tile_pool(name="wops", bufs=2, space="PSUM")
                wops = wops_ctx.__enter__()

                def wo_block(tb):
                    pso = [wops.tile([P, 512], F32, tag=f"po{eo}", name=f"po{eo}")
                           for eo in range(2)]
                    for eo in range(2):
                        for ec in range(4):
                            nc.tensor.matmul(pso[eo][:],
                                             OTT[ec][:, tb * P:(tb + 1) * P],
                                             WoT_t[ec][:, eo * 512:(eo + 1) * 512],
                                             start=(ec == 0), stop=(ec == 3))
                    ot_ = ost.tile([P, E], F32, tag="ot", name="ot")
                    nc.vector.tensor_copy(ot_[:, 0:512], pso[0][:])
                    nc.scalar.activation(ot_[:, 512:1024], pso[1][:], AF.Copy)
                    nc.sync.dma_start(
                        out_v[tb // 8, (tb % 8) * P:(tb % 8 + 1) * P, :], ot_[:])

                # 4-bank iDFT passes (oh x e4-pair) so Wo for the even half
                # overlaps the odd-half iDFT on the PE
                for oh in range(2):
                    nm_c, nm_s = ("iec", "ies") if oh == 0 else ("ioc", "ios")
                    mct = []
                    mst = []
                    for jt in range(5):
                        msz = P if jt < 4 else 1
                        r0 = jt * P
                        mc = gsl.tile([P, 1024], R, tag=f"gc{jt}", name=f"gc{jt}", bufs=2)
                        ms = gsl.tile([P, 1024], R, tag=f"gs{jt}", name=f"gs{jt}", bufs=2)
                        nc.sync.dma_start(mc[0:msz, :], ig_d[nm_c].ap()[r0:r0 + msz, :])
                        nc.sync.dma_start(ms[0:msz, :], ig_d[nm_s].ap()[r0:r0 + msz, :])
                        mct.append(mc)
                        mst.append(ms)
                    for ep in range(2):
                        pst = [[idps.tile([P, 512], F32, tag=f"ph{i}_{t2}", name=f"ph{i}_{t2}")
                                for t2 in range(2)] for i in range(2)]
                        for jt in range(5):
                            msz = P if jt < 4 else 1
                            mc = mct[jt]
                            ms = mst[jt]
                            if jt < 4:
                                sr = Ur[jt] if oh == 0 else Vr[jt]
                                si = Ui[jt] if oh == 0 else Vi[jt]
                            else:
                                sr, si = our[4], oui[4]
                            for i in range(2):
                                e4 = 2 * ep + i
                                for t2 in range(2):
                                    nc.tensor.matmul(pst[i][t2][:],
                                                     sr[0:msz, e4 * P:(e4 + 1) * P],
                                                     mc[0:msz, t2 * 512:(t2 + 1) * 512],
                                                     start=(jt == 0), stop=False)
                                    nc.tensor.matmul(pst[i][t2][:],
                                                     si[0:msz, e4 * P:(e4 + 1) * P],
                                                     ms[0:msz, t2 * 512:(t2 + 1) * 512],
                                                     start=False, stop=(jt == 4))
                        for i in range(2):
                            e4 = 2 * ep + i
                            for t2 in range(2):
                                dst = OTT[e4][:, oh * 1024 + t2 * 512:oh * 1024 + (t2 + 1) * 512]
                                if i == 0:
                                    nc.vector.tensor_copy(dst, pst[i][t2][:])
                                else:
                                    nc.scalar.activation(dst, pst[i][t2][:], AF.Copy)
                    if oh == 0:
                        for tb in range(8):
                            wo_block(tb)
                for tb in range(8, LT):
                    wo_block(tb)
                wops_ctx.__exit__(None, None, None)
                idps_ctx.__exit__(None, None, None)

    nc.finalize()
    return nc


def kernel(**inputs):
    from concourse.bass_utils import run_bass_kernel_spmd

    rdt = ml_dtypes.bfloat16 if MM_BF16 else np.float32
    if "nc" not in _CACHE:
        _CACHE["nc"] = _build()
        Ce, Se, Co, So, Ec, Es, Oc, Os = _dft_consts()
        _CACHE["consts"] = {
            "ce": _pretile_half(Ce).astype(rdt),
            "se": _pretile_half(Se).astype(rdt),
            "co": _pretile_half(Co).astype(rdt),
            "so": _pretile_half(So).astype(rdt),
            "iec": Ec.astype(rdt), "ies": Es.astype(rdt),
            "ioc": Oc.astype(rdt), "ios": Os.astype(rdt),
        }
    nc = _CACHE["nc"]
    C = _CACHE["consts"]

    perm = np.r_[0:L:2, 1:L:2]
    q = np.ascontiguousarray(inputs["query"], dtype=np.float32)[:, perm, :]
    kv = np.asarray(inputs["key_value"], dtype=np.float32)[:, perm, :]
    gamma = np.asarray(inputs["gamma"], np.float32)
    beta = np.asarray(inputs["beta"], np.float32)
    in_maps = []
    for core in range(8):
        b = core // 2
        hg = core % 2
        cs = slice(hg * 512, (hg + 1) * 512)
        kvt = np.ascontiguousarray(
            kv[b].reshape(LT, P, ET, P).transpose(2, 1, 0, 3).reshape(ET * P, LT * P)
        ).astype(rdt)
        m = {
            "q": np.ascontiguousarray(q[b]).astype(rdt),
            "kvt": kvt,
            "ce": C["ce"], "se": C["se"], "co": C["co"], "so": C["so"],
            "iec": C["iec"], "ies": C["ies"], "ioc": C["ioc"], "ios": C["ios"],
            "WoT": np.ascontiguousarray(inputs["Wo"][:, cs].T.astype(rdt)),
        }
        for nm in ("qr", "qi", "kr", "ki", "vr", "vi"):
            Wcs = np.asarray(inputs["W" + nm], np.float32)[cs, :]
            if nm in ("qr", "qi"):
                m[f"dc{nm}"] = np.ascontiguousarray(
                    (Wcs @ beta) * math.sqrt(L), np.float32).reshape(512, 1)
                Wcs = Wcs * gamma[None, :]
            m[f"W{nm}"] = np.ascontiguousarray(Wcs.T.astype(rdt))
            m[f"b{nm}"] = np.ascontiguousarray(inputs["b" + nm][cs], np.float32).reshape(512, 1)
        in_maps.append(m)

    res = run_bass_kernel_spmd(nc, in_maps, core_ids=list(range(8)))
    _CACHE["last"] = res
    out = np.empty((B, L, E), np.float32)
    for b in range(B):
        out[b] = res.results[2 * b]["out"] + res.results[2 * b + 1]["out"]
    return out
